# revision 1
# baseline (speedup 1.0000x reference)
"""Trainium2 Bass kernel for nn_ClipOTLoss (CLIP-style OT/Sinkhorn loss).

Computes, for full inputs features[B,D], prototypes[K,D], logits[B,K]:
    w = normalize(prototypes, axis=1)
    sims = features @ w.T / TEMPERATURE
    soft_code = sinkhorn(sims)            (3 iters, eps=0.7)
    loss = -mean_b sum_k soft_code * log_softmax(logits)

Distribution: data-parallel over B across 8 NeuronCores; prototypes
replicated; the Sinkhorn row-marginal (sum over B per prototype k)
is a 16KB AllReduce per iteration.

Key algebraic structure exploited: Sinkhorn preserves diagonal scaling,
Q = E * A[k] * Bb[b] with E = exp(sims/eps), so no [B,K] matrix is ever
rewritten -- each iteration is two matvecs on the TensorEngine plus tiny
per-vector updates.  Also sum_k soft_code == 1 exactly, so
    loss_b = LSE(logits_b) - (1/s_b) * sum_k E*A*logits_b,
    s_b = sum_k E[b,k]*A[k]
and log_probs is never materialized.
"""

import os
import sys

import numpy as np

sys.path.insert(0, "/opt/trn_rl_repo")

import concourse.bass as bass  # noqa: E402
import concourse.bacc as bacc  # noqa: E402
import concourse.tile as tile  # noqa: E402
import concourse.mybir as mybir  # noqa: E402
from concourse.masks import make_identity  # noqa: E402

F32 = mybir.dt.float32
BF16 = mybir.dt.bfloat16
AF = mybir.ActivationFunctionType
ALU = mybir.AluOpType

TEMPERATURE = 0.01
EPSILON = 0.7
NUM_ITERS = 3
TINY = 1e-8

P = 128  # partitions
NSLICE = 512  # max matmul free dim (one PSUM bank of f32)


def build_nc(B_loc=1024, K=4096, D=1024, n_cores=8):
    NB = B_loc // P  # number of 128-row b-blocks per core
    NK = K // P  # number of 128-wide k-chunks
    ND = D // P  # number of 128-deep d-chunks
    KH = K // 2  # half of K (psum half-tile for main matmul)
    exp_scale = 1.0 / (TEMPERATURE * EPSILON)
    r_marg = 1.0 / K
    c_marg = 1.0 / (B_loc * n_cores)
    loss_scale = 1.0 / (B_loc * n_cores)
    groups = list(range(0, NK, 4))  # k-tile groups of 4 for transposes
    rg = [list(range(n_cores))]

    nc = bacc.Bacc(None, target_bir_lowering=False, debug=False)

    feats = nc.declare_dram_parameter("features", [B_loc, D], F32, isOutput=False)
    protos = nc.declare_dram_parameter("prototypes", [K, D], F32, isOutput=False)
    logits = nc.declare_dram_parameter("logits", [B_loc, K], F32, isOutput=False)
    out_ext = nc.declare_dram_parameter("out", [1], F32, isOutput=True)

    # collective bounce buffers (internal DRAM; outputs must be Shared)
    m_in_d = [nc.dram_tensor(f"cc_m_in{i}", [K], F32) for i in range(NUM_ITERS)]
    m_out_d = [
        nc.dram_tensor(f"cc_m_out{i}", [K], F32, addr_space="Shared")
        for i in range(NUM_ITERS)
    ]
    l_in_d = nc.dram_tensor("cc_l_in", [8], F32)
    l_out_d = nc.dram_tensor("cc_l_out", [8], F32, addr_space="Shared")

    with tile.TileContext(nc) as tc:
        with (
            tc.tile_pool(name="single", bufs=1) as single,
            tc.tile_pool(name="big", bufs=1) as bigp,
            tc.tile_pool(name="stage", bufs=4) as stage,
            tc.tile_pool(name="wsc", bufs=4) as wscp,
            tc.tile_pool(name="lg", bufs=2) as lgp,
            tc.tile_pool(name="ps", bufs=2, space="PSUM") as psp,
        ):
            # ---- packed small-tensor arenas (SBUF slots pad to 4KB; do
            # not waste a slot per tiny vector) ----
            smf_cols = 320 + 4 * NK + 18 * NB
            smf = single.tile([P, smf_cols], F32, tag="smf")
            smb = single.tile([P, 384 + NK + NB + 16], BF16, tag="smb")

            class _Cols:
                def __init__(self, t):
                    self.t, self.off = t, 0

                def take(self, np_, nf):
                    ap = self.t[:np_, self.off : self.off + nf]
                    self.off += nf
                    return ap

            cf, cb = _Cols(smf), _Cols(smb)

            ident_f = cf.take(P, P)
            make_identity(nc, ident_f)
            ones_f = cf.take(P, 1)
            nc.vector.memset(ones_f, 1.0)
            norm2 = cf.take(P, NK)
            rn = cf.take(P, NK)
            tmpk = cf.take(P, NK)
            cs_fl = cf.take(P, NB * 2)
            se_fl = cf.take(P, NB * 4)
            dot_fl = cf.take(P, NB * 4)
            A_v = cf.take(P, NK)
            Bb_v = cf.take(P, NB)
            tmpb = cf.take(P, NB)
            cs0 = cf.take(P, NB)
            se_s = cf.take(P, NB)
            lse = cf.take(P, NB)
            dot_s = cf.take(P, NB)
            rs = cf.take(P, NB)
            losses = cf.take(P, NB)
            lcol = cf.take(P, 1)
            mg_sb = cf.take(NK, P)
            loss_sb = cf.take(1, 8)
            lg_sb = cf.take(1, 8)

            ident_b = cb.take(P, P)
            make_identity(nc, ident_b)
            ones_b = cb.take(1, P)
            nc.vector.memset(ones_b, 1.0)
            A_cm = cb.take(P, NK)
            Bb_cm = cb.take(P, NB)
            at_sb = cb.take(NK, P)  # A^T [NK, 128]

            at_flat = single.tile([1, K], BF16, tag="atflat")  # A as one row
            vec_sb = single.tile([1, K], F32, tag="vecsb")  # m / v staging

            # ---- persistent big tensors ----
            E = bigp.tile([P, NB, K], BF16, tag="E")  # E[b,k], b-major
            wn_t = bigp.tile([P, ND, K], BF16, tag="bigA")  # w_norm^T [d,k]
            f_t = bigp.tile([P, ND, B_loc], BF16, tag="ft")  # features^T [d,b]

            # =========================================================
            # Prologue A: prototypes -> normalized, bf16, transposed
            # =========================================================
            for g in groups:
                ws_tiles = []
                for kt in range(g, g + 4):
                    wt = stage.tile([P, D], F32, tag="stage")
                    nc.sync.dma_start(out=wt[:], in_=protos[kt * P : (kt + 1) * P, :])
                    # squared row norms (fused square+row-sum on ScalarE);
                    # the Square output goes to the ws tile, which the
                    # tensor_scalar below overwrites anyway.
                    ws = wscp.tile([P, D], BF16, tag="wsc")
                    nc.scalar.activation(
                        out=ws[:], in_=wt[:], func=AF.Square,
                        accum_out=norm2[:, kt : kt + 1],
                    )
                    # rn = exp(-0.5 * ln(norm2))  (avoids sqrt table switch)
                    nc.scalar.activation(
                        out=tmpk[:, kt : kt + 1], in_=norm2[:, kt : kt + 1], func=AF.Ln,
                    )
                    nc.scalar.activation(
                        out=rn[:, kt : kt + 1], in_=tmpk[:, kt : kt + 1], func=AF.Exp,
                        scale=-0.5,
                    )
                    # scale rows, cast to bf16 (overwrites the Square scratch)
                    nc.vector.tensor_scalar(
                        out=ws[:], in0=wt[:], scalar1=rn[:, kt : kt + 1], scalar2=None,
                        op0=ALU.mult,
                    )
                    ws_tiles.append(ws)
                # transpose the 4 k-tiles into wn_t columns
                for j in range(ND):
                    tp = psp.tile([P, 4 * P], BF16, tag="big")
                    for q in range(4):
                        nc.tensor.transpose(
                            tp[:, q * P : (q + 1) * P],
                            ws_tiles[q][:, j * P : (j + 1) * P],
                            ident_b[:],
                        )
                    if j % 2 == 0:
                        nc.scalar.copy(out=wn_t[:, j, g * P : (g + 4) * P], in_=tp[:])
                    else:
                        nc.vector.tensor_copy(
                            out=wn_t[:, j, g * P : (g + 4) * P], in_=tp[:]
                        )

            # =========================================================
            # Prologue B: features -> bf16 transposed [d, b]
            # =========================================================
            for g in range(0, NB, 4):
                gf = min(4, NB - g)
                ftiles = []
                for c in range(g, g + gf):
                    ft_in = stage.tile([P, D], F32, tag="stage")
                    nc.sync.dma_start(out=ft_in[:], in_=feats[c * P : (c + 1) * P, :])
                    ftiles.append(ft_in)
                for j in range(ND):
                    tp = psp.tile([P, 4 * P], F32, tag="big")
                    for q in range(gf):
                        nc.tensor.transpose(
                            tp[:, q * P : (q + 1) * P],
                            ftiles[q][:, j * P : (j + 1) * P],
                            ident_f[:],
                        )
                    if j % 2 == 0:
                        nc.scalar.copy(
                            out=f_t[:, j, g * P : (g + gf) * P], in_=tp[:, : gf * P]
                        )
                    else:
                        nc.vector.tensor_copy(
                            out=f_t[:, j, g * P : (g + gf) * P], in_=tp[:, : gf * P]
                        )

            # =========================================================
            # Main matmul: sims_raw = f @ wn^T, E = exp(scale*sims_raw)
            # per b-block c, per K-half h: psum [128, KH]
            # =========================================================
            for c in range(NB):
                for h in range(2):
                    mm_ps = psp.tile([P, KH], F32, tag="big")
                    for j in range(ND):
                        for n in range(KH // NSLICE):
                            nc.tensor.matmul(
                                mm_ps[:, n * NSLICE : (n + 1) * NSLICE],
                                f_t[:, j, c * P : (c + 1) * P],
                                wn_t[:, j, h * KH + n * NSLICE : h * KH + (n + 1) * NSLICE],
                                start=(j == 0),
                                stop=(j == ND - 1),
                            )
                    # exp (+ fused row-sum partial for colsum0)
                    nc.scalar.activation(
                        out=E[:, c, h * KH : (h + 1) * KH],
                        in_=mm_ps[:],
                        func=AF.Exp,
                        scale=exp_scale,
                        accum_out=cs_fl[:, c * 2 + h : c * 2 + h + 1],
                    )

            # =========================================================
            # LSE of logits (streamed; exp in-place; fused row-sums)
            # =========================================================
            KQ = K // 4
            for c in range(NB):
                for h in range(4):
                    lt = lgp.tile([P, KQ], F32, tag="lg")
                    nc.sync.dma_start(
                        out=lt[:],
                        in_=logits[c * P : (c + 1) * P, h * KQ : (h + 1) * KQ],
                    )
                    nc.scalar.activation(
                        out=lt[:], in_=lt[:], func=AF.Exp,
                        accum_out=se_fl[:, c * 4 + h : c * 4 + h + 1],
                    )
            se_q = se_fl.rearrange("p (c q) -> p c q", q=4)
            nc.vector.tensor_reduce(
                out=se_s, in_=se_q, axis=mybir.AxisListType.X, op=ALU.add
            )
            nc.scalar.activation(out=lse, in_=se_s, func=AF.Ln)

            # colsum0 -> Bb0 = 1/colsum0
            cs_pair = cs_fl.rearrange("p (c two) -> p c two", two=2)
            nc.vector.tensor_tensor(
                out=cs0, in0=cs_pair[:, :, 0], in1=cs_pair[:, :, 1], op=ALU.add
            )
            nc.vector.reciprocal(out=Bb_v, in_=cs0)
            nc.vector.tensor_copy(out=Bb_cm, in_=Bb_v)
            nc.vector.memset(A_v, 1.0)

            # =========================================================
            # E^T (k-major) for the v-direction matvecs.  Emitted lazily
            # inside iteration 1's AllReduce gap so the PE stays busy
            # (and HAM-warm) while the collective is in flight.
            # =========================================================
            ET = bigp.tile([P, NK, B_loc], BF16, tag="bigA")  # reuses wn_t slot

            def emit_ET():
                for j in range(NK):
                    for g in range(0, NB, 4):
                        gf = min(4, NB - g)
                        tp = psp.tile([P, 4 * P], BF16, tag="big")
                        for q in range(gf):
                            nc.tensor.transpose(
                                tp[:, q * P : (q + 1) * P],
                                E[:, g + q, j * P : (j + 1) * P],
                                ident_b[:],
                            )
                        if j % 2 == 0:
                            nc.scalar.copy(
                                out=ET[:, j, g * P : (g + gf) * P], in_=tp[:, : gf * P]
                            )
                        else:
                            nc.vector.tensor_copy(
                                out=ET[:, j, g * P : (g + gf) * P], in_=tp[:, : gf * P]
                            )

            # =========================================================
            # Sinkhorn iterations (factorized)
            # =========================================================
            for it in range(NUM_ITERS):
                # ---- u-direction: m[k] = sum_b E[b,k] * Bb[b]  (PE matvec)
                for half in range(2):
                    mv_ps = psp.tile([1, KH], F32, tag="big")
                    for n in range(KH // NSLICE):
                        for c in range(NB):
                            nc.tensor.matmul(
                                mv_ps[:1, n * NSLICE : (n + 1) * NSLICE],
                                Bb_cm[:, c : c + 1],
                                E[:, c, half * KH + n * NSLICE : half * KH + (n + 1) * NSLICE],
                                start=(c == 0),
                                stop=(c == NB - 1),
                            )
                    nc.scalar.copy(
                        out=vec_sb[:1, half * KH : (half + 1) * KH], in_=mv_ps[:1, :]
                    )
                nc.sync.dma_start(out=m_in_d[it][:], in_=vec_sb[:1, :K])
                nc.gpsimd.collective_compute(
                    "AllReduce",
                    ALU.add,
                    replica_groups=rg,
                    ins=[m_in_d[it][:]],
                    outs=[m_out_d[it][:]],
                )
                if it == 0:
                    # E^T transposes fill iteration 1's AllReduce gap
                    emit_ET()
                else:
                    # PE warmers: keep the HAM clock hot through the
                    # AllReduce gap (dead transposes into a scratch bank)
                    warm = psp.tile([P, P], BF16, tag="big")
                    for _ in range(48):
                        nc.tensor.transpose(warm[:, :], ident_b[:], ident_b[:])
                nc.sync.dma_start(
                    out=mg_sb[:], in_=m_out_d[it][:].rearrange("(a b) -> a b", a=NK)
                )
                # redistribute m to column-major [128, NK]
                mg_ps = psp.tile([P, NK], F32, tag="big")
                nc.tensor.transpose(mg_ps[:, :], mg_sb[:], ident_f[:NK, :NK])
                # A update: A *= r / (A*m + TINY)
                nc.vector.tensor_tensor(
                    out=tmpk[:], in0=A_v[:], in1=mg_ps[:], op=ALU.mult
                )
                nc.vector.tensor_scalar(
                    out=tmpk[:], in0=tmpk[:], scalar1=TINY, scalar2=None, op0=ALU.add
                )
                nc.vector.reciprocal(out=tmpk[:], in_=tmpk[:])
                nc.vector.tensor_tensor(
                    out=A_v[:], in0=A_v[:], in1=tmpk[:], op=ALU.mult
                )
                nc.vector.tensor_scalar(
                    out=A_v[:], in0=A_v[:], scalar1=r_marg, scalar2=None, op0=ALU.mult
                )
                nc.vector.tensor_copy(out=A_cm[:], in_=A_v[:])

                # ---- v-direction: v[b] = Bb[b] * sum_k E^T[k,b] * A[k]
                # Iteration 3's v-step only rescales Bb, which cancels in
                # the final per-sample normalization -- skip it.
                if it < NUM_ITERS - 1:
                    vv_ps = psp.tile([1, B_loc], F32, tag="big")
                    for n in range(B_loc // NSLICE):
                        for j in range(NK):
                            nc.tensor.matmul(
                                vv_ps[:1, n * NSLICE : (n + 1) * NSLICE],
                                A_cm[:, j : j + 1],
                                ET[:, j, n * NSLICE : (n + 1) * NSLICE],
                                start=(j == 0),
                                stop=(j == NK - 1),
                            )
                    nc.scalar.copy(out=vec_sb[:1, :B_loc], in_=vv_ps[:1, :])
                    # redistribute v to column-major [128, NB]
                    vc_ps = psp.tile([P, NB], F32, tag="big")
                    for c in range(NB):
                        nc.tensor.transpose(
                            vc_ps[:, c : c + 1],
                            vec_sb[:1, c * P : (c + 1) * P],
                            ident_f[:1, :1],
                        )
                    # Bb update: Bb *= c / (Bb*t + TINY)
                    nc.vector.tensor_tensor(
                        out=tmpb[:], in0=Bb_v[:], in1=vc_ps[:], op=ALU.mult
                    )
                    nc.vector.tensor_scalar(
                        out=tmpb[:], in0=tmpb[:], scalar1=TINY, scalar2=None, op0=ALU.add
                    )
                    nc.vector.reciprocal(out=tmpb[:], in_=tmpb[:])
                    nc.vector.tensor_tensor(
                        out=Bb_v[:], in0=Bb_v[:], in1=tmpb[:], op=ALU.mult
                    )
                    nc.vector.tensor_scalar(
                        out=Bb_v[:], in0=Bb_v[:], scalar1=c_marg, scalar2=None, op0=ALU.mult
                    )
                    nc.vector.tensor_copy(out=Bb_cm[:], in_=Bb_v[:])

            # =========================================================
            # Final: s[b] = sum_k E^T A (matvec), then loss pieces
            # =========================================================
            sv_ps = psp.tile([1, B_loc], F32, tag="big")
            for n in range(B_loc // NSLICE):
                for j in range(NK):
                    nc.tensor.matmul(
                        sv_ps[:1, n * NSLICE : (n + 1) * NSLICE],
                        A_cm[:, j : j + 1],
                        ET[:, j, n * NSLICE : (n + 1) * NSLICE],
                        start=(j == 0),
                        stop=(j == NK - 1),
                    )
            nc.scalar.copy(out=vec_sb[:1, :B_loc], in_=sv_ps[:1, :])
            sc_ps = psp.tile([P, NB], F32, tag="big")
            for c in range(NB):
                nc.tensor.transpose(
                    sc_ps[:, c : c + 1],
                    vec_sb[:1, c * P : (c + 1) * P],
                    ident_f[:1, :1],
                )
            nc.vector.reciprocal(out=rs[:], in_=sc_ps[:])

            # A broadcast along partitions: A_bc[p, k] = A[k] (bf16)
            at_ps = psp.tile([NK, P], BF16, tag="big")
            nc.tensor.transpose(at_ps[:NK, :], A_cm[:], ident_b[:])
            nc.scalar.copy(out=at_sb[:NK, :], in_=at_ps[:NK, :])
            nc.sync.dma_start(out=at_flat[:1, :], in_=at_sb[:])
            A_bc = bigp.tile([P, K], BF16, tag="ft")  # reuses f_t slot
            for g in range(0, NK, 4):
                gk = min(4, NK - g)
                bc_ps = psp.tile([P, 4 * P], F32, tag="big")
                for q in range(gk):
                    nc.tensor.matmul(
                        bc_ps[:, q * P : (q + 1) * P],
                        ones_b[:1, :],
                        at_flat[:1, (g + q) * P : (g + q + 1) * P],
                        start=True,
                        stop=True,
                    )
                nc.vector.tensor_copy(
                    out=A_bc[:, g * P : (g + gk) * P], in_=bc_ps[:, : gk * P]
                )

            # EA = E * A_bc (in place), then dot pass over logits
            for c in range(NB):
                nc.vector.tensor_tensor(
                    out=E[:, c, :], in0=E[:, c, :], in1=A_bc[:], op=ALU.mult
                )
            for c in range(NB):
                for h in range(4):
                    lt = lgp.tile([P, KQ], F32, tag="lg")
                    nc.sync.dma_start(
                        out=lt[:],
                        in_=logits[c * P : (c + 1) * P, h * KQ : (h + 1) * KQ],
                    )
                    nc.vector.scalar_tensor_tensor(
                        out=lt[:],
                        in0=E[:, c, h * KQ : (h + 1) * KQ],
                        scalar=rs[:, c : c + 1],
                        in1=lt[:],
                        op0=ALU.mult,
                        op1=ALU.mult,
                        accum_out=dot_fl[:, c * 4 + h : c * 4 + h + 1],
                    )
            dot_q = dot_fl.rearrange("p (c q) -> p c q", q=4)
            nc.vector.tensor_reduce(
                out=dot_s, in_=dot_q, axis=mybir.AxisListType.X, op=ALU.add
            )

            # losses = LSE - dot ; partial = sum over local samples
            nc.vector.tensor_tensor(
                out=losses, in0=lse, in1=dot_s, op=ALU.subtract
            )
            nc.vector.tensor_reduce(
                out=lcol, in_=losses, axis=mybir.AxisListType.X, op=ALU.add
            )
            lp_ps = psp.tile([1, 1], F32, tag="big")
            nc.tensor.matmul(
                lp_ps[:1, :1], ones_f[:, :1], lcol[:, :1], start=True, stop=True
            )
            nc.vector.memset(loss_sb[:], 0.0)
            nc.scalar.activation(
                out=loss_sb[:1, 0:1], in_=lp_ps[:1, :1], func=AF.Copy,
                scale=loss_scale,
            )
            nc.sync.dma_start(out=l_in_d[:], in_=loss_sb[:1, :])
            nc.gpsimd.collective_compute(
                "AllReduce",
                ALU.add,
                replica_groups=rg,
                ins=[l_in_d[:]],
                outs=[l_out_d[:]],
            )
            nc.sync.dma_start(out=lg_sb[:1, :], in_=l_out_d[:])
            nc.sync.dma_start(out=out_ext[:], in_=lg_sb[:1, 0:1])

    nc.compile()
    return nc


LAST_RESULT = None


def kernel(features, prototypes, logits):
    from concourse.bass_utils import run_bass_kernel_spmd

    global LAST_RESULT
    n_cores = 8
    B, D = features.shape
    K = prototypes.shape[0]
    B_loc = B // n_cores

    nc = build_nc(B_loc=B_loc, K=K, D=D, n_cores=n_cores)

    features = np.ascontiguousarray(features, dtype=np.float32)
    prototypes = np.ascontiguousarray(prototypes, dtype=np.float32)
    logits = np.ascontiguousarray(logits, dtype=np.float32)

    in_maps = [
        {
            "features": features[i * B_loc : (i + 1) * B_loc],
            "prototypes": prototypes,
            "logits": logits[i * B_loc : (i + 1) * B_loc],
        }
        for i in range(n_cores)
    ]
    res = run_bass_kernel_spmd(
        nc,
        in_maps,
        list(range(n_cores)),
        trace=bool(os.environ.get("CLIP_OT_TRACE")),
    )
    LAST_RESULT = res
    return np.asarray(res.results[0]["out"], dtype=np.float32).reshape(())



# revision 25
# speedup vs baseline: 1.2356x; 1.2356x over previous
"""Trainium2 Bass kernel for nn_ClipOTLoss (CLIP-style OT/Sinkhorn loss).

Computes, for full inputs features[B,D], prototypes[K,D], logits[B,K]:
    w = normalize(prototypes, axis=1)
    sims = features @ w.T / TEMPERATURE
    soft_code = sinkhorn(sims)            (3 iters, eps=0.7)
    loss = -mean_b sum_k soft_code * log_softmax(logits)

Distribution: data-parallel over B across 8 NeuronCores; prototypes
replicated; the Sinkhorn row-marginal (sum over B per prototype k)
is a 16KB AllReduce per iteration.  Per-core partial losses are summed
on the host (no final AllReduce).

Structure (v2):
  - Sinkhorn preserves diagonal scaling: Q = E * A[k] * Bb[b] with
    E = exp(sims/eps); each iteration is one PE matvec (u-direction,
    partition-reduction over b) plus one DVE free-dim reduction
    (v-direction, against a GpSimd-broadcast A row).  No E^T is ever
    built.
  - Prologue keeps the Scalar engine on one activation table
    (Square/Sqrt) instead of thrashing Square/Ln/Exp per k-tile.
  - The iteration-0 u-matvec accumulates inside the main matmul phase,
    so the first AllReduce fires right after the last exp.
  - logits are streamed exactly once, during the AllReduce gaps:
    Scalar does exp+accum (for LSE), Vector writes EL = E*logits bf16
    into the slot vacated by w^T.  The epilogue is two DVE reduce
    sweeps:  s_b = sum_k E*A,  dotraw_b = sum_k EL*A,
    loss_b = LSE_b - dotraw_b/s_b.
"""

import os
import sys

import numpy as np

sys.path.insert(0, "/opt/trn_rl_repo")

import concourse.bass as bass  # noqa: E402
import concourse.bacc as bacc  # noqa: E402
import concourse.tile as tile  # noqa: E402
import concourse.mybir as mybir  # noqa: E402
from concourse.masks import make_identity  # noqa: E402

F32 = mybir.dt.float32
BF16 = mybir.dt.bfloat16
AF = mybir.ActivationFunctionType
ALU = mybir.AluOpType

TEMPERATURE = 0.01
EPSILON = 0.7
NUM_ITERS = 3
TINY = 1e-8

P = 128  # partitions
NSLICE = 512  # max matmul free dim (one PSUM bank of f32)


def build_nc(B_loc=1024, K=4096, D=1024, n_cores=8):
    NB = B_loc // P  # number of 128-row b-blocks per core
    NK = K // P  # number of 128-wide k-chunks
    ND = D // P  # number of 128-deep d-chunks
    exp_scale = 1.0 / (TEMPERATURE * EPSILON)
    r_marg = 1.0 / K
    c_marg = 1.0 / (B_loc * n_cores)
    loss_scale = 1.0 / (B_loc * n_cores)
    rg = [list(range(n_cores))]
    WARM_AR = bool(int(os.environ.get('CLIP_OT_WARM_AR', '1')))
    NWARM = 110  # PE clock warmers per AllReduce gap

    nc = bacc.Bacc(None, target_bir_lowering=False, debug=False)

    feats = nc.declare_dram_parameter("features", [B_loc, D], F32, isOutput=False)
    protos = nc.declare_dram_parameter("prototypes", [K, D], F32, isOutput=False)
    logits = nc.declare_dram_parameter("logits", [B_loc, K], F32, isOutput=False)
    out_ext = nc.declare_dram_parameter("out", [1], F32, isOutput=True)

    # collective bounce buffers (internal DRAM; outputs must be Shared)
    m_in_d = [nc.dram_tensor(f"cc_m_in{i}", [K], F32) for i in range(NUM_ITERS)]
    m_out_d = [
        nc.dram_tensor(f"cc_m_out{i}", [K], F32, addr_space="Shared")
        for i in range(NUM_ITERS)
    ]
    w_in_d = nc.dram_tensor("cc_w_in", [8], F32)
    w_out_d = nc.dram_tensor("cc_w_out", [8], F32, addr_space="Shared")

    with tile.TileContext(nc) as tc:
        with (
            tc.tile_pool(name="single", bufs=1) as single,
            tc.tile_pool(name="big", bufs=1) as bigp,
            tc.tile_pool(name="stage", bufs=4) as stage,
            tc.tile_pool(name="wsc", bufs=4) as wscp,
            tc.tile_pool(name="psmm", bufs=2, space="PSUM") as psmm,
            tc.tile_pool(name="psmv", bufs=2, space="PSUM") as psmv,
        ):
            # ---- packed small-tensor arenas ----
            smf = single.tile([P, 256 + 128 + NK * 3 + NB * 24], F32, tag="smf")
            smb = single.tile([P, 128 + NB + 8], BF16, tag="smb")

            class _Cols:
                def __init__(self, t):
                    self.t, self.off = t, 0

                def take(self, np_, nf):
                    ap = self.t[:np_, self.off : self.off + nf]
                    self.off += nf
                    return ap

            cf, cb = _Cols(smf), _Cols(smb)

            ident_f = cf.take(P, P)
            make_identity(nc, ident_f)
            ones_f = cf.take(P, 1)
            nc.vector.memset(ones_f, 1.0)
            norm2 = cf.take(P, NK)
            sqrtn = cf.take(P, NK)
            rn = cf.take(P, NK)
            cs_fl = cf.take(P, NB * 8)  # per-eighth colsum partials
            cs0 = cf.take(P, NB)
            Bb = cf.take(P, NB)
            bt = cf.take(P, NB)
            se_fl = cf.take(P, NB * 4)  # per-quarter sum(exp(logits))
            se_s = cf.take(P, NB)
            lse = cf.take(P, NB)
            s_col = cf.take(P, NB)
            dotraw = cf.take(P, NB)
            rs = cf.take(P, NB)
            dots = cf.take(P, NB)
            losses = cf.take(P, NB)
            lcol = cf.take(P, 1)
            loss_sb = cf.take(1, 8)
            warm_src = cf.take(P, 8)

            ident_b = cb.take(P, P)
            make_identity(nc, ident_b)
            Bb_bf = cb.take(P, NB)

            # [32, 128]-layout iteration state: x[a, b] = x[k = a*128 + b]
            m32 = [single.tile([NK, P], F32, tag=f"m32_{i}", name=f"m32_{i}") for i in range(NUM_ITERS)]
            A32 = single.tile([NK, P], F32, tag="A32")
            A32t = single.tile([NK, P], F32, tag="A32t")
            A32bf = single.tile([NK, P], BF16, tag="A32bf")
            at_flat = single.tile([1, K], BF16, tag="atflat")
            A_bc = single.tile([P, K], BF16, tag="Abc")
            msb = single.tile([1, K], F32, tag="msb")  # m staging row
            warm_sb = single.tile([1, 8], F32, tag="warmsb")

            # ---- persistent big tensors ----
            E = bigp.tile([P, NB, K], BF16, tag="E")  # E[b,k], b-major
            wn_t = bigp.tile([P, ND, K], BF16, tag="W")  # w_norm^T [d,k]
            f_t = bigp.tile([P, ND, B_loc], BF16, tag="F")  # features^T [d,b]

            # =========================================================
            # Warm-up AllReduce: absorbs the CC-stream startup latency
            # so the first real AllReduce triggers immediately.
            # =========================================================
            if WARM_AR:
                nc.vector.memset(warm_sb[:], 0.0)
                nc.sync.dma_start(out=w_in_d[:], in_=warm_sb[:1, :])
                nc.gpsimd.collective_compute(
                    "AllReduce", ALU.add, replica_groups=rg,
                    ins=[w_in_d[:]], outs=[w_out_d[:]],
                )

            # =========================================================
            # Prologue A: prototypes -> normalized, bf16, transposed.
            # Scalar stays on the Sqrt table (Square lives in every
            # table) -- one ACT_TABLE_LOAD for the whole prologue.
            # =========================================================
            for g in range(0, NK, 4):
                ws_tiles = []
                for kt in range(g, g + 4):
                    wt = stage.tile([P, D], F32, tag="stage")
                    nc.sync.dma_start(out=wt[:], in_=protos[kt * P : (kt + 1) * P, :])
                    ws = wscp.tile([P, D], BF16, tag="wsc")
                    nc.scalar.activation(
                        out=ws[:], in_=wt[:], func=AF.Square,
                        accum_out=norm2[:, kt : kt + 1],
                    )
                    ws_tiles.append((wt, ws))
                nc.scalar.sqrt(out=sqrtn[:, g : g + 4], in_=norm2[:, g : g + 4])
                nc.vector.reciprocal(out=rn[:, g : g + 4], in_=sqrtn[:, g : g + 4])
                for q, (wt, ws) in enumerate(ws_tiles):
                    kt = g + q
                    # scale rows by 1/||w||, cast to bf16 (overwrites Square scratch)
                    nc.vector.tensor_scalar(
                        out=ws[:], in0=wt[:], scalar1=rn[:, kt : kt + 1], scalar2=None,
                        op0=ALU.mult,
                    )
                # transpose the 4 k-tiles into wn_t columns
                for j in range(ND):
                    tp = psmm.tile([P, 4 * P], BF16, tag="mm")
                    for q in range(4):
                        nc.tensor.transpose(
                            tp[:, q * P : (q + 1) * P],
                            ws_tiles[q][1][:, j * P : (j + 1) * P],
                            ident_b[:],
                        )
                    if j % 2 == 0:
                        nc.scalar.copy(out=wn_t[:, j, g * P : (g + 4) * P], in_=tp[:])
                    else:
                        nc.vector.tensor_copy(
                            out=wn_t[:, j, g * P : (g + 4) * P], in_=tp[:]
                        )

            # =========================================================
            # Prologue B: features -> bf16 (DVE cast), transposed [d, b]
            # =========================================================
            for g in range(0, NB, 4):
                gf = min(4, NB - g)
                ftiles = []
                for c in range(g, g + gf):
                    ft_in = stage.tile([P, D], F32, tag="stage")
                    nc.sync.dma_start(out=ft_in[:], in_=feats[c * P : (c + 1) * P, :])
                    fb = wscp.tile([P, D], BF16, tag="wsc")
                    nc.vector.tensor_copy(out=fb[:], in_=ft_in[:])
                    ftiles.append(fb)
                for j in range(ND):
                    tp = psmm.tile([P, 4 * P], BF16, tag="mm")
                    for q in range(gf):
                        nc.tensor.transpose(
                            tp[:, q * P : (q + 1) * P],
                            ftiles[q][:, j * P : (j + 1) * P],
                            ident_b[:],
                        )
                    if j % 2 == 0:
                        nc.scalar.copy(
                            out=f_t[:, j, g * P : (g + gf) * P], in_=tp[:, : gf * P]
                        )
                    else:
                        nc.vector.tensor_copy(
                            out=f_t[:, j, g * P : (g + gf) * P], in_=tp[:, : gf * P]
                        )

            # =========================================================
            # Main matmul: sims_raw = f @ wn^T, E = exp(scale*sims_raw)
            # per b-block c, per 512-col k-slice: psum [128, 512].
            # The iteration-0 u-matvec (m0[k] = sum_b E[b,k]*Bb0[b])
            # accumulates into two [1, K/2] PSUM tiles as blocks finish,
            # so the first AllReduce fires right after the last exp.
            # =========================================================
            KQ4 = K // 4  # u-matvec accumulates in [1, 1024] PSUM quarters

            def mv_quarters(pfx):
                return [
                    psmv.tile([1, KQ4], F32, tag=f"mvq{q}", bufs=1, name=f"{pfx}_{q}")
                    for q in range(3)
                ]

            def emit_mv_fused(c, qt, first, last):
                # quarters 0-2, accumulated block-by-block
                for q in range(3):
                    for n in range(2):
                        o = q * KQ4 + n * NSLICE
                        nc.tensor.matmul(
                            qt[q][:1, n * NSLICE : (n + 1) * NSLICE],
                            Bb_bf[:, c : c + 1],
                            E[:, c, o : o + NSLICE],
                            start=first,
                            stop=last,
                        )

            def emit_mv_tail(it, qt, pfx):
                # copy quarters 0-2 to the staging row (parallel engines),
                # then quarter 3 reuses PSUM slot 0, then DMA + AR
                nc.scalar.copy(out=msb[:1, 0:KQ4], in_=qt[0][:1, :])
                nc.vector.tensor_copy(out=msb[:1, KQ4 : 2 * KQ4], in_=qt[1][:1, :])
                nc.scalar.copy(out=msb[:1, 2 * KQ4 : 3 * KQ4], in_=qt[2][:1, :])
                q3 = psmv.tile([1, KQ4], F32, tag="mvq0", bufs=1, name=f"{pfx}_3")
                for c in range(NB):
                    for n in range(2):
                        o = 3 * KQ4 + n * NSLICE
                        nc.tensor.matmul(
                            q3[:1, n * NSLICE : (n + 1) * NSLICE],
                            Bb_bf[:, c : c + 1],
                            E[:, c, o : o + NSLICE],
                            start=(c == 0),
                            stop=(c == NB - 1),
                        )
                nc.vector.tensor_copy(out=msb[:1, 3 * KQ4 :], in_=q3[:1, :])
                nc.sync.dma_start(out=m_in_d[it][:], in_=msb[:1, :])
                nc.gpsimd.collective_compute(
                    "AllReduce", ALU.add, replica_groups=rg,
                    ins=[m_in_d[it][:]], outs=[m_out_d[it][:]],
                )

            mv0 = mv_quarters("mv0")
            for c in range(NB):
                for e in range(K // NSLICE):
                    mm_ps = psmm.tile([P, NSLICE], F32, tag="mm")
                    for j in range(ND):
                        nc.tensor.matmul(
                            mm_ps[:],
                            f_t[:, j, c * P : (c + 1) * P],
                            wn_t[:, j, e * NSLICE : (e + 1) * NSLICE],
                            start=(j == 0),
                            stop=(j == ND - 1),
                        )
                    nc.scalar.activation(
                        out=E[:, c, e * NSLICE : (e + 1) * NSLICE],
                        in_=mm_ps[:],
                        func=AF.Exp,
                        scale=exp_scale,
                        accum_out=cs_fl[:, c * 8 + e : c * 8 + e + 1],
                    )
                # Bb0 for block c: 1 / sum_k E[b,k]
                nc.vector.tensor_reduce(
                    out=cs0[:, c : c + 1],
                    in_=cs_fl[:, c * 8 : (c + 1) * 8].rearrange("p (a q) -> p a q", a=1),
                    axis=mybir.AxisListType.X,
                    op=ALU.add,
                )
                nc.vector.reciprocal(out=Bb[:, c : c + 1], in_=cs0[:, c : c + 1])
                nc.vector.tensor_copy(out=Bb_bf[:, c : c + 1], in_=Bb[:, c : c + 1])
                if c >= 1:
                    emit_mv_fused(c - 1, mv0, first=(c - 1 == 0), last=False)
            emit_mv_fused(NB - 1, mv0, first=False, last=True)
            emit_mv_tail(0, mv0, "mv0")

            # =========================================================
            # logits stream: LSE partials (Scalar) + EL = E*logits bf16
            # (Vector) into the slot vacated by wn_t.  Split across the
            # AllReduce gaps.  EL[b,k] reuses tag "W".
            # =========================================================
            KL = K // 4  # logits stream tile width
            EL = bigp.tile([P, NB, K], BF16, tag="W")
            vscr = bigp.tile([P, K], BF16, tag="F")  # v-sweep dump, aliases f_t

            def emit_logits(c):
                for q in range(4):
                    lt = stage.tile([P, KL], F32, tag="stage")
                    nc.sync.dma_start(
                        out=lt[:],
                        in_=logits[c * P : (c + 1) * P, q * KL : (q + 1) * KL],
                    )
                    sexp = wscp.tile([P, KL], BF16, tag="wsc")
                    nc.scalar.activation(
                        out=sexp[:], in_=lt[:], func=AF.Exp,
                        accum_out=se_fl[:, c * 4 + q : c * 4 + q + 1],
                    )
                    nc.vector.tensor_tensor(
                        out=EL[:, c, q * KL : (q + 1) * KL],
                        in0=E[:, c, q * KL : (q + 1) * KL],
                        in1=lt[:],
                        op=ALU.mult,
                    )

            def emit_warmers(n):
                # dependency-free PE busy-work: keeps the clock governor
                # at full p-state through an AllReduce gap
                wp = psmm.tile([P, P], BF16, tag="mm")
                for _ in range(n):
                    nc.tensor.transpose(wp[:, :], ident_b[:], ident_b[:])

            def emit_gated_warmers(it, n):
                # warmers that depend on the AllReduce result: they run
                # right before the u-matvec, bridging the A-update gap
                wp = psmm.tile([P, NK], F32, tag="mm")
                for _ in range(n):
                    nc.tensor.transpose(wp[:, :NK], m32[it][:, :], ident_f[:NK, :NK])

            def emit_A_update(it):
                # m arrives as [32, 128]; A update stays in that layout.
                nc.sync.dma_start(
                    out=m32[it][:], in_=m_out_d[it][:].rearrange("(a b) -> a b", a=NK)
                )
                if it < NUM_ITERS - 1:
                    emit_gated_warmers(it, 40)
                if it == 0:
                    nc.vector.tensor_scalar(
                        out=A32t[:], in0=m32[it][:], scalar1=TINY, scalar2=None,
                        op0=ALU.add,
                    )
                    nc.vector.reciprocal(out=A32[:], in_=A32t[:])
                    nc.vector.tensor_scalar(
                        out=A32[:], in0=A32[:], scalar1=r_marg, scalar2=None,
                        op0=ALU.mult,
                    )
                else:
                    nc.vector.tensor_tensor(
                        out=A32t[:], in0=A32[:], in1=m32[it][:], op=ALU.mult
                    )
                    nc.vector.tensor_scalar(
                        out=A32t[:], in0=A32t[:], scalar1=TINY, scalar2=None,
                        op0=ALU.add,
                    )
                    nc.vector.reciprocal(out=A32t[:], in_=A32t[:])
                    nc.vector.tensor_tensor(
                        out=A32[:], in0=A32[:], in1=A32t[:], op=ALU.mult
                    )
                    nc.vector.tensor_scalar(
                        out=A32[:], in0=A32[:], scalar1=r_marg, scalar2=None,
                        op0=ALU.mult,
                    )
                nc.vector.tensor_copy(out=A32bf[:], in_=A32[:])
                nc.sync.dma_start(out=at_flat[:1, :], in_=A32bf[:])
                nc.gpsimd.partition_broadcast(A_bc[:], at_flat[:1, :])

            # =========================================================
            # Sinkhorn iterations
            # =========================================================
            for it in range(NUM_ITERS):
                if it == 0:
                    emit_warmers(NWARM)
                    for c in range(4):
                        emit_logits(c)
                emit_A_update(it)
                if it < NUM_ITERS - 1:
                    # v(c) -> Bb(c) -> u(c), pipelined per block
                    mv = mv_quarters(f"mv{it + 1}")
                    for c in range(NB):
                        # accum = sum_k (E*Bb)*A = Bb * v_raw, in one op
                        nc.vector.scalar_tensor_tensor(
                            out=vscr[:],
                            in0=E[:, c, :],
                            scalar=Bb[:, c : c + 1],
                            in1=A_bc[:],
                            op0=ALU.mult,
                            op1=ALU.mult,
                            accum_out=bt[:, c : c + 1],
                        )
                        # Bb *= c_marg / (Bb*v_raw + TINY), column c only
                        nc.vector.tensor_scalar(
                            out=bt[:, c : c + 1], in0=bt[:, c : c + 1],
                            scalar1=TINY, scalar2=None, op0=ALU.add,
                        )
                        nc.vector.reciprocal(out=bt[:, c : c + 1], in_=bt[:, c : c + 1])
                        nc.vector.tensor_tensor(
                            out=Bb[:, c : c + 1], in0=Bb[:, c : c + 1],
                            in1=bt[:, c : c + 1], op=ALU.mult,
                        )
                        nc.vector.tensor_scalar(
                            out=Bb[:, c : c + 1], in0=Bb[:, c : c + 1],
                            scalar1=c_marg, scalar2=None, op0=ALU.mult,
                        )
                        nc.vector.tensor_copy(
                            out=Bb_bf[:, c : c + 1], in_=Bb[:, c : c + 1]
                        )
                        emit_mv_fused(c, mv, first=(c == 0), last=(c == NB - 1))
                    emit_mv_tail(it + 1, mv, f"mv{it + 1}")
                    if it == 0:
                        emit_warmers(NWARM)
                        for c in range(4, 6):
                            emit_logits(c)
                    else:
                        for c in range(6, NB):
                            emit_logits(c)

            # =========================================================
            # Final: s_b = sum_k E*A, dotraw_b = sum_k EL*A,
            # loss_b = LSE_b - dotraw_b / s_b
            # =========================================================
            for c in range(NB):
                nc.vector.scalar_tensor_tensor(
                    out=vscr[:], in0=E[:, c, :], scalar=1.0, in1=A_bc[:],
                    op0=ALU.mult, op1=ALU.mult,
                    accum_out=s_col[:, c : c + 1],
                )
                nc.vector.scalar_tensor_tensor(
                    out=vscr[:], in0=EL[:, c, :], scalar=1.0, in1=A_bc[:],
                    op0=ALU.mult, op1=ALU.mult,
                    accum_out=dotraw[:, c : c + 1],
                )
            se_q = se_fl.rearrange("p (c q) -> p c q", q=4)
            nc.vector.tensor_reduce(
                out=se_s, in_=se_q, axis=mybir.AxisListType.X, op=ALU.add
            )
            nc.scalar.activation(out=lse, in_=se_s, func=AF.Ln)
            nc.vector.reciprocal(out=rs[:], in_=s_col[:])
            nc.vector.tensor_tensor(out=dots, in0=dotraw, in1=rs, op=ALU.mult)
            nc.vector.tensor_tensor(out=losses, in0=lse, in1=dots, op=ALU.subtract)
            nc.vector.tensor_reduce(
                out=lcol, in_=losses, axis=mybir.AxisListType.X, op=ALU.add
            )
            lp_ps = psmm.tile([1, NSLICE], F32, tag="mm")
            nc.tensor.matmul(
                lp_ps[:1, :1], ones_f[:, :1], lcol[:, :1], start=True, stop=True
            )
            nc.scalar.activation(
                out=loss_sb[:1, 0:1], in_=lp_ps[:1, :1], func=AF.Copy,
                scale=loss_scale,
            )
            nc.sync.dma_start(out=out_ext[:], in_=loss_sb[:1, 0:1])

    nc.compile()
    return nc


LAST_RESULT = None


def kernel(features, prototypes, logits):
    from concourse.bass_utils import run_bass_kernel_spmd

    global LAST_RESULT
    n_cores = 8
    B, D = features.shape
    K = prototypes.shape[0]
    B_loc = B // n_cores

    nc = build_nc(B_loc=B_loc, K=K, D=D, n_cores=n_cores)

    features = np.ascontiguousarray(features, dtype=np.float32)
    prototypes = np.ascontiguousarray(prototypes, dtype=np.float32)
    logits = np.ascontiguousarray(logits, dtype=np.float32)

    in_maps = [
        {
            "features": features[i * B_loc : (i + 1) * B_loc],
            "prototypes": prototypes,
            "logits": logits[i * B_loc : (i + 1) * B_loc],
        }
        for i in range(n_cores)
    ]
    res = run_bass_kernel_spmd(
        nc,
        in_maps,
        list(range(n_cores)),
        trace=bool(os.environ.get("CLIP_OT_TRACE")),
    )
    LAST_RESULT = res
    total = 0.0
    for i in range(n_cores):
        total += float(np.asarray(res.results[i]["out"], dtype=np.float64)[0])
    return np.float32(total)


# revision 34
# speedup vs baseline: 1.3534x; 1.0953x over previous
"""Trainium2 Bass kernel for nn_ClipOTLoss (CLIP-style OT/Sinkhorn loss).

Computes, for full inputs features[B,D], prototypes[K,D], logits[B,K]:
    w = normalize(prototypes, axis=1)
    sims = features @ w.T / TEMPERATURE
    soft_code = sinkhorn(sims)            (3 iters, eps=0.7)
    loss = -mean_b sum_k soft_code * log_softmax(logits)

Distribution: data-parallel over B across 8 NeuronCores; prototypes
replicated; the Sinkhorn row-marginal (sum over B per prototype k)
is a 16KB AllReduce per iteration.  Per-core partial losses are summed
on the host (no final AllReduce).

Structure (v2):
  - Sinkhorn preserves diagonal scaling: Q = E * A[k] * Bb[b] with
    E = exp(sims/eps); each iteration is one PE matvec (u-direction,
    partition-reduction over b) plus one DVE free-dim reduction
    (v-direction, against a GpSimd-broadcast A row).  No E^T is ever
    built.
  - Prologue keeps the Scalar engine on one activation table
    (Square/Sqrt) instead of thrashing Square/Ln/Exp per k-tile.
  - The iteration-0 u-matvec accumulates inside the main matmul phase,
    so the first AllReduce fires right after the last exp.
  - logits are streamed exactly once, during the AllReduce gaps:
    Scalar does exp+accum (for LSE), Vector writes EL = E*logits bf16
    into the slot vacated by w^T.  The epilogue is two DVE reduce
    sweeps:  s_b = sum_k E*A,  dotraw_b = sum_k EL*A,
    loss_b = LSE_b - dotraw_b/s_b.
"""

import os
import sys

import numpy as np

sys.path.insert(0, "/opt/trn_rl_repo")

import concourse.bass as bass  # noqa: E402
import concourse.bacc as bacc  # noqa: E402
import concourse.tile as tile  # noqa: E402
import concourse.mybir as mybir  # noqa: E402
from concourse.masks import make_identity  # noqa: E402

F32 = mybir.dt.float32
BF16 = mybir.dt.bfloat16
AF = mybir.ActivationFunctionType
ALU = mybir.AluOpType

TEMPERATURE = 0.01
EPSILON = 0.7
NUM_ITERS = 3
TINY = 1e-8

P = 128  # partitions
NSLICE = 512  # max matmul free dim (one PSUM bank of f32)


def build_nc(B_loc=1024, K=4096, D=1024, n_cores=8):
    NB = B_loc // P  # number of 128-row b-blocks per core
    NK = K // P  # number of 128-wide k-chunks
    ND = D // P  # number of 128-deep d-chunks
    exp_scale = 1.0 / (TEMPERATURE * EPSILON)
    r_marg = 1.0 / K
    c_marg = 1.0 / (B_loc * n_cores)
    loss_scale = 1.0 / (B_loc * n_cores)
    rg = [list(range(n_cores))]
    WARM_AR = bool(int(os.environ.get('CLIP_OT_WARM_AR', '1')))
    NWARM = 110  # PE clock warmers per AllReduce gap

    nc = bacc.Bacc(None, target_bir_lowering=False, debug=False)

    feats = nc.declare_dram_parameter("features", [B_loc, D], F32, isOutput=False)
    protos = nc.declare_dram_parameter("prototypes", [K, D], F32, isOutput=False)
    logits = nc.declare_dram_parameter("logits", [B_loc, K], F32, isOutput=False)
    out_ext = nc.declare_dram_parameter("out", [1], F32, isOutput=True)

    # collective bounce buffers (internal DRAM; outputs must be Shared)
    m_in_d = [nc.dram_tensor(f"cc_m_in{i}", [K], F32) for i in range(NUM_ITERS)]
    m_out_d = [
        nc.dram_tensor(f"cc_m_out{i}", [K], F32, addr_space="Shared")
        for i in range(NUM_ITERS)
    ]
    w_in_d = nc.dram_tensor("cc_w_in", [8], F32)
    w_out_d = nc.dram_tensor("cc_w_out", [8], F32, addr_space="Shared")

    with tile.TileContext(nc) as tc:
        with (
            tc.tile_pool(name="single", bufs=1) as single,
            tc.tile_pool(name="big", bufs=1) as bigp,
            tc.tile_pool(name="stage", bufs=4) as stage,
            tc.tile_pool(name="wsc", bufs=4) as wscp,
            tc.tile_pool(name="psmm", bufs=2, space="PSUM") as psmm,
            tc.tile_pool(name="psmv", bufs=2, space="PSUM") as psmv,
        ):
            # ---- packed small-tensor arenas ----
            smf = single.tile([P, 256 + 128 + NK * 3 + NB * 40], F32, tag="smf")
            smb = single.tile([P, 256 + NB + 8], BF16, tag="smb")

            class _Cols:
                def __init__(self, t):
                    self.t, self.off = t, 0

                def take(self, np_, nf):
                    ap = self.t[:np_, self.off : self.off + nf]
                    self.off += nf
                    return ap

            cf, cb = _Cols(smf), _Cols(smb)

            ident_f = cf.take(P, P)
            make_identity(nc, ident_f)
            ones_f = cf.take(P, 1)
            nc.vector.memset(ones_f, 1.0)
            norm2 = cf.take(P, NK)
            sqrtn = cf.take(P, NK)
            rn = cf.take(P, NK)
            cs_fl = cf.take(P, NB * 8)  # per-eighth colsum partials
            cs0 = cf.take(P, NB)
            Bb = cf.take(P, NB)
            bt = cf.take(P, NB)
            se_fl = cf.take(P, NB * 4)  # per-quarter sum(exp(logits))
            se_s = cf.take(P, NB)
            lse = cf.take(P, NB)
            s_col = cf.take(P, NB)
            dotraw = cf.take(P, NB)
            rs = cf.take(P, NB)
            dots = cf.take(P, NB)
            losses = cf.take(P, NB)
            lcol = cf.take(P, 1)
            loss_sb = cf.take(1, 8)
            warm_src = cf.take(P, 8)
            s4 = cf.take(P, NB * 4)  # tail quarter-partials (s)
            d4 = cf.take(P, NB * 4)  # tail quarter-partials (dot)
            v4 = cf.take(P, NB * 4)  # v-sweep quarter-partials

            ident_b = cb.take(P, P)
            make_identity(nc, ident_b)
            Bb_bf = cb.take(P, NB)
            ones_b1 = cb.take(1, P)
            nc.vector.memset(ones_b1, 1.0)

            # [32, 128]-layout iteration state: x[a, b] = x[k = a*128 + b]
            m32 = [single.tile([NK, P], F32, tag=f"m32_{i}", name=f"m32_{i}") for i in range(NUM_ITERS)]
            A32 = single.tile([NK, P], F32, tag="A32")
            A32t = single.tile([NK, P], F32, tag="A32t")
            A32bf = single.tile([NK, P], BF16, tag="A32bf")
            at_flat = single.tile([1, K], BF16, tag="atflat")
            A_bc = single.tile([P, K], BF16, tag="Abc")
            msb = single.tile([1, K], F32, tag="msb")  # m staging row
            warm_sb = single.tile([1, 8], F32, tag="warmsb")

            # ---- persistent big tensors ----
            E = bigp.tile([P, NB, K], BF16, tag="E")  # E[b,k], b-major
            wn_t = bigp.tile([P, ND, K], BF16, tag="W")  # w_norm^T [d,k]
            f_t = bigp.tile([P, ND, B_loc], BF16, tag="F")  # features^T [d,b]

            # =========================================================
            # Warm-up AllReduce: absorbs the CC-stream startup latency
            # so the first real AllReduce triggers immediately.
            # =========================================================
            if WARM_AR:
                nc.vector.memset(warm_sb[:], 0.0)
                nc.sync.dma_start(out=w_in_d[:], in_=warm_sb[:1, :])
                nc.gpsimd.collective_compute(
                    "AllReduce", ALU.add, replica_groups=rg,
                    ins=[w_in_d[:]], outs=[w_out_d[:]],
                )

            # =========================================================
            # Prologue A: prototypes -> normalized, bf16, transposed.
            # Scalar stays on the Sqrt table (Square lives in every
            # table) -- one ACT_TABLE_LOAD for the whole prologue.
            # =========================================================
            for kt in range(NK):
                wt = stage.tile([P, D], F32, tag="stage")
                nc.sync.dma_start(out=wt[:], in_=protos[kt * P : (kt + 1) * P, :])
                ws = wscp.tile([P, D], BF16, tag="wsc")
                nc.scalar.activation(
                    out=ws[:], in_=wt[:], func=AF.Square,
                    accum_out=norm2[:, kt : kt + 1],
                )
                nc.scalar.sqrt(out=sqrtn[:, kt : kt + 1], in_=norm2[:, kt : kt + 1])
                nc.vector.reciprocal(out=rn[:, kt : kt + 1], in_=sqrtn[:, kt : kt + 1])
                # scale rows by 1/||w||, cast to bf16 (overwrites Square scratch)
                nc.vector.tensor_scalar(
                    out=ws[:], in0=wt[:], scalar1=rn[:, kt : kt + 1], scalar2=None,
                    op0=ALU.mult,
                )
                # transpose the tile's 8 d-chunks into one PSUM bank, then
                # one strided copy into the wn_t column
                tp = psmm.tile([P, ND * P], BF16, tag="mm")
                for j in range(ND):
                    nc.tensor.transpose(
                        tp[:, j * P : (j + 1) * P],
                        ws[:, j * P : (j + 1) * P],
                        ident_b[:],
                    )
                if kt % 2 == 0:
                    nc.scalar.copy(
                        out=wn_t[:, :, kt * P : (kt + 1) * P],
                        in_=tp[:].rearrange("p (j b) -> p j b", j=ND),
                    )
                else:
                    nc.vector.tensor_copy(
                        out=wn_t[:, :, kt * P : (kt + 1) * P],
                        in_=tp[:].rearrange("p (j b) -> p j b", j=ND),
                    )

            # =========================================================
            # Prologue B: features -> bf16 (DVE cast), transposed [d, b]
            # =========================================================
            for c in range(NB):
                ft_in = stage.tile([P, D], F32, tag="stage")
                nc.sync.dma_start(out=ft_in[:], in_=feats[c * P : (c + 1) * P, :])
                fb = wscp.tile([P, D], BF16, tag="wsc")
                nc.vector.tensor_copy(out=fb[:], in_=ft_in[:])
                tp = psmm.tile([P, ND * P], BF16, tag="mm")
                for j in range(ND):
                    nc.tensor.transpose(
                        tp[:, j * P : (j + 1) * P],
                        fb[:, j * P : (j + 1) * P],
                        ident_b[:],
                    )
                if c % 2 == 0:
                    nc.scalar.copy(
                        out=f_t[:, :, c * P : (c + 1) * P],
                        in_=tp[:].rearrange("p (j b) -> p j b", j=ND),
                    )
                else:
                    nc.vector.tensor_copy(
                        out=f_t[:, :, c * P : (c + 1) * P],
                        in_=tp[:].rearrange("p (j b) -> p j b", j=ND),
                    )

            # =========================================================
            # Main matmul: sims_raw = f @ wn^T, E = exp(scale*sims_raw)
            # per b-block c, per 512-col k-slice: psum [128, 512].
            # The iteration-0 u-matvec (m0[k] = sum_b E[b,k]*Bb0[b])
            # accumulates into two [1, K/2] PSUM tiles as blocks finish,
            # so the first AllReduce fires right after the last exp.
            # =========================================================
            KQ4 = K // 4  # u-matvec accumulates in [1, 1024] PSUM quarters

            def mv_quarters(pfx):
                return [
                    psmv.tile([1, KQ4], F32, tag=f"mvq{q}", bufs=1, name=f"{pfx}_{q}")
                    for q in range(3)
                ]

            def emit_mv_fused(c, qt, first, last):
                # quarters 0-2, accumulated block-by-block
                for q in range(3):
                    for n in range(2):
                        o = q * KQ4 + n * NSLICE
                        nc.tensor.matmul(
                            qt[q][:1, n * NSLICE : (n + 1) * NSLICE],
                            Bb_bf[:, c : c + 1],
                            E[:, c, o : o + NSLICE],
                            start=first,
                            stop=last,
                        )

            def emit_mv_tail(it, qt, pfx):
                # copy quarters 0-2 to the staging row (parallel engines),
                # then quarter 3 reuses PSUM slot 0, then DMA + AR
                nc.scalar.copy(out=msb[:1, 0:KQ4], in_=qt[0][:1, :])
                nc.vector.tensor_copy(out=msb[:1, KQ4 : 2 * KQ4], in_=qt[1][:1, :])
                nc.scalar.copy(out=msb[:1, 2 * KQ4 : 3 * KQ4], in_=qt[2][:1, :])
                q3 = psmv.tile([1, KQ4], F32, tag="mvq0", bufs=1, name=f"{pfx}_3")
                for c in range(NB):
                    for n in range(2):
                        o = 3 * KQ4 + n * NSLICE
                        nc.tensor.matmul(
                            q3[:1, n * NSLICE : (n + 1) * NSLICE],
                            Bb_bf[:, c : c + 1],
                            E[:, c, o : o + NSLICE],
                            start=(c == 0),
                            stop=(c == NB - 1),
                        )
                nc.vector.tensor_copy(out=msb[:1, 3 * KQ4 :], in_=q3[:1, :])
                nc.sync.dma_start(out=m_in_d[it][:], in_=msb[:1, :])
                nc.gpsimd.collective_compute(
                    "AllReduce", ALU.add, replica_groups=rg,
                    ins=[m_in_d[it][:]], outs=[m_out_d[it][:]],
                )

            mv0 = mv_quarters("mv0")
            for c in range(NB):
                for e in range(K // NSLICE):
                    mm_ps = psmm.tile([P, NSLICE], F32, tag="mm")
                    for j in range(ND):
                        nc.tensor.matmul(
                            mm_ps[:],
                            f_t[:, j, c * P : (c + 1) * P],
                            wn_t[:, j, e * NSLICE : (e + 1) * NSLICE],
                            start=(j == 0),
                            stop=(j == ND - 1),
                        )
                    nc.scalar.activation(
                        out=E[:, c, e * NSLICE : (e + 1) * NSLICE],
                        in_=mm_ps[:],
                        func=AF.Exp,
                        scale=exp_scale,
                        accum_out=cs_fl[:, c * 8 + e : c * 8 + e + 1],
                    )
                # Bb0 for block c: 1 / sum_k E[b,k]
                nc.vector.tensor_reduce(
                    out=cs0[:, c : c + 1],
                    in_=cs_fl[:, c * 8 : (c + 1) * 8].rearrange("p (a q) -> p a q", a=1),
                    axis=mybir.AxisListType.X,
                    op=ALU.add,
                )
                nc.vector.reciprocal(out=Bb[:, c : c + 1], in_=cs0[:, c : c + 1])
                nc.vector.tensor_copy(out=Bb_bf[:, c : c + 1], in_=Bb[:, c : c + 1])
                if c >= 1:
                    emit_mv_fused(c - 1, mv0, first=(c - 1 == 0), last=False)
            emit_mv_fused(NB - 1, mv0, first=False, last=True)
            emit_mv_tail(0, mv0, "mv0")

            # =========================================================
            # logits stream: LSE partials (Scalar) + EL = E*logits bf16
            # (Vector) into the slot vacated by wn_t.  Split across the
            # AllReduce gaps.  EL[b,k] reuses tag "W".
            # =========================================================
            KL = K // 4  # logits stream tile width
            EL = bigp.tile([P, NB, K], BF16, tag="W")
            vscr = bigp.tile([P, K], BF16, tag="F")  # v-sweep dump, aliases f_t

            def emit_logits(c):
                for q in range(4):
                    lt = stage.tile([P, KL], F32, tag="stage")
                    nc.sync.dma_start(
                        out=lt[:],
                        in_=logits[c * P : (c + 1) * P, q * KL : (q + 1) * KL],
                    )
                    sexp = wscp.tile([P, KL], BF16, tag="wsc")
                    nc.scalar.activation(
                        out=sexp[:], in_=lt[:], func=AF.Exp,
                        accum_out=se_fl[:, c * 4 + q : c * 4 + q + 1],
                    )
                    nc.vector.tensor_tensor(
                        out=EL[:, c, q * KL : (q + 1) * KL],
                        in0=E[:, c, q * KL : (q + 1) * KL],
                        in1=lt[:],
                        op=ALU.mult,
                    )

            def emit_warmers(n):
                # dependency-free PE busy-work: keeps the clock governor
                # at full p-state through an AllReduce gap
                wp = psmm.tile([P, P], BF16, tag="mm")
                for _ in range(n):
                    nc.tensor.transpose(wp[:, :], ident_b[:], ident_b[:])

            def emit_gated_warmers(it, n):
                # warmers that depend on the AllReduce result: they run
                # right before the u-matvec, bridging the A-update gap
                wp = psmm.tile([P, NK], F32, tag="mm")
                for _ in range(n):
                    nc.tensor.transpose(wp[:, :NK], m32[it][:, :], ident_f[:NK, :NK])

            def emit_A_update(it):
                # m arrives as [32, 128]; A update stays in that layout.
                nc.sync.dma_start(
                    out=m32[it][:], in_=m_out_d[it][:].rearrange("(a b) -> a b", a=NK)
                )
                if it < NUM_ITERS - 1:
                    emit_gated_warmers(it, 40)
                if it == 0:
                    nc.vector.tensor_scalar(
                        out=A32t[:], in0=m32[it][:], scalar1=TINY, scalar2=None,
                        op0=ALU.add,
                    )
                    nc.vector.reciprocal(out=A32[:], in_=A32t[:])
                    nc.vector.tensor_scalar(
                        out=A32[:], in0=A32[:], scalar1=r_marg, scalar2=None,
                        op0=ALU.mult,
                    )
                else:
                    nc.vector.tensor_tensor(
                        out=A32t[:], in0=A32[:], in1=m32[it][:], op=ALU.mult
                    )
                    nc.vector.tensor_scalar(
                        out=A32t[:], in0=A32t[:], scalar1=TINY, scalar2=None,
                        op0=ALU.add,
                    )
                    nc.vector.reciprocal(out=A32t[:], in_=A32t[:])
                    nc.vector.tensor_tensor(
                        out=A32[:], in0=A32[:], in1=A32t[:], op=ALU.mult
                    )
                    nc.vector.tensor_scalar(
                        out=A32[:], in0=A32[:], scalar1=r_marg, scalar2=None,
                        op0=ALU.mult,
                    )
                nc.vector.tensor_copy(out=A32bf[:], in_=A32[:])
                nc.sync.dma_start(out=at_flat[:1, :], in_=A32bf[:])
                # broadcast the A row to all partitions on the (idle) PE:
                # ones[1,128]^T @ at_flat[1,512] -> [128,512] per slice
                for n in range(K // NSLICE):
                    bc_ps = psmm.tile([P, NSLICE], F32, tag="mm")
                    nc.tensor.matmul(
                        bc_ps[:],
                        ones_b1[:1, :],
                        at_flat[:1, n * NSLICE : (n + 1) * NSLICE],
                        start=True,
                        stop=True,
                    )
                    if n % 2 == 0:
                        nc.scalar.copy(
                            out=A_bc[:, n * NSLICE : (n + 1) * NSLICE], in_=bc_ps[:]
                        )
                    else:
                        nc.vector.tensor_copy(
                            out=A_bc[:, n * NSLICE : (n + 1) * NSLICE], in_=bc_ps[:]
                        )

            # =========================================================
            # Sinkhorn iterations
            # =========================================================
            for it in range(NUM_ITERS):
                if it == 0:
                    emit_warmers(NWARM)
                    for c in range(3):
                        emit_logits(c)
                emit_A_update(it)
                if it < NUM_ITERS - 1:
                    # v(c) -> Bb(c) -> u(c), pipelined per block.  Blocks
                    # 0-1 use one DVE STT each (accum = Bb*v_raw); blocks
                    # 2-7 use DVE TT quarters reduced on the Scalar engine
                    # (engines balanced, u starts as soon as block 0 lands).
                    mv = mv_quarters(f"mv{it + 1}")

                    def finish_bb(c, folded):
                        if not folded:
                            # bt currently holds v_raw; fold Bb in
                            nc.vector.tensor_tensor(
                                out=bt[:, c : c + 1], in0=Bb[:, c : c + 1],
                                in1=bt[:, c : c + 1], op=ALU.mult,
                            )
                        nc.vector.tensor_scalar(
                            out=bt[:, c : c + 1], in0=bt[:, c : c + 1],
                            scalar1=TINY, scalar2=None, op0=ALU.add,
                        )
                        nc.vector.reciprocal(out=bt[:, c : c + 1], in_=bt[:, c : c + 1])
                        nc.vector.tensor_tensor(
                            out=Bb[:, c : c + 1], in0=Bb[:, c : c + 1],
                            in1=bt[:, c : c + 1], op=ALU.mult,
                        )
                        nc.vector.tensor_scalar(
                            out=Bb[:, c : c + 1], in0=Bb[:, c : c + 1],
                            scalar1=c_marg, scalar2=None, op0=ALU.mult,
                        )
                        nc.vector.tensor_copy(
                            out=Bb_bf[:, c : c + 1], in_=Bb[:, c : c + 1]
                        )
                        emit_mv_fused(c, mv, first=(c == 0), last=(c == NB - 1))

                    for c in range(2):
                        # accum = sum_k (E*Bb)*A = Bb * v_raw, in one op
                        nc.vector.scalar_tensor_tensor(
                            out=vscr[:],
                            in0=E[:, c, :],
                            scalar=Bb[:, c : c + 1],
                            in1=A_bc[:],
                            op0=ALU.mult,
                            op1=ALU.mult,
                            accum_out=bt[:, c : c + 1],
                        )
                        finish_bb(c, folded=True)
                    def reduce_v4(c):
                        nc.vector.tensor_reduce(
                            out=bt[:, c : c + 1],
                            in_=v4[:, c * 4 : (c + 1) * 4].rearrange(
                                "p (a q) -> p a q", a=1
                            ),
                            axis=mybir.AxisListType.X,
                            op=ALU.add,
                        )

                    # stagger: emit TTs of block c, then the (Scalar-fed)
                    # reduce of block c-1, so the DVE never waits on Scalar
                    for c in range(2, NB):
                        for q in range(4):
                            vq = wscp.tile([P, KQ4], BF16, tag="wsc")
                            nc.vector.tensor_tensor(
                                out=vq[:],
                                in0=E[:, c, q * KQ4 : (q + 1) * KQ4],
                                in1=A_bc[:, q * KQ4 : (q + 1) * KQ4],
                                op=ALU.mult,
                            )
                            nc.scalar.activation(
                                out=vq[:], in_=vq[:], func=AF.Copy,
                                accum_out=v4[:, c * 4 + q : c * 4 + q + 1],
                            )
                        if c > 2:
                            reduce_v4(c - 1)
                            finish_bb(c - 1, folded=False)
                    reduce_v4(NB - 1)
                    finish_bb(NB - 1, folded=False)
                    emit_mv_tail(it + 1, mv, f"mv{it + 1}")
                    if it == 0:
                        emit_warmers(NWARM)
                        for c in range(3, 6):
                            emit_logits(c)
                    else:
                        for c in range(6, NB):
                            emit_logits(c)

            # =========================================================
            # Final: s_b = sum_k E*A, dotraw_b = sum_k EL*A,
            # loss_b = LSE_b - dotraw_b / s_b
            # =========================================================
            # blocks 2-7: DVE TT quarters reduced on Scalar (runs both
            # engines); blocks 0-1: DVE STTs at the end (Scalar drains)
            for c in range(2, NB):
                for src, part in ((E, s4), (EL, d4)):
                    for q in range(4):
                        tq = wscp.tile([P, KQ4], BF16, tag="wsc")
                        nc.vector.tensor_tensor(
                            out=tq[:],
                            in0=src[:, c, q * KQ4 : (q + 1) * KQ4],
                            in1=A_bc[:, q * KQ4 : (q + 1) * KQ4],
                            op=ALU.mult,
                        )
                        nc.scalar.activation(
                            out=tq[:], in_=tq[:], func=AF.Copy,
                            accum_out=part[:, c * 4 + q : c * 4 + q + 1],
                        )
            for c in range(2):
                nc.vector.scalar_tensor_tensor(
                    out=vscr[:], in0=E[:, c, :], scalar=1.0, in1=A_bc[:],
                    op0=ALU.mult, op1=ALU.mult,
                    accum_out=s_col[:, c : c + 1],
                )
                nc.vector.scalar_tensor_tensor(
                    out=vscr[:], in0=EL[:, c, :], scalar=1.0, in1=A_bc[:],
                    op0=ALU.mult, op1=ALU.mult,
                    accum_out=dotraw[:, c : c + 1],
                )
            for c in range(2, NB):
                for part, dst in ((s4, s_col), (d4, dotraw)):
                    nc.vector.tensor_reduce(
                        out=dst[:, c : c + 1],
                        in_=part[:, c * 4 : (c + 1) * 4].rearrange(
                            "p (a q) -> p a q", a=1
                        ),
                        axis=mybir.AxisListType.X,
                        op=ALU.add,
                    )
            se_q = se_fl.rearrange("p (c q) -> p c q", q=4)
            nc.vector.tensor_reduce(
                out=se_s, in_=se_q, axis=mybir.AxisListType.X, op=ALU.add
            )
            nc.scalar.activation(out=lse, in_=se_s, func=AF.Ln)
            nc.vector.reciprocal(out=rs[:], in_=s_col[:])
            nc.vector.tensor_tensor(out=dots, in0=dotraw, in1=rs, op=ALU.mult)
            nc.vector.tensor_tensor(out=losses, in0=lse, in1=dots, op=ALU.subtract)
            nc.vector.tensor_reduce(
                out=lcol, in_=losses, axis=mybir.AxisListType.X, op=ALU.add
            )
            lp_ps = psmm.tile([1, NSLICE], F32, tag="mm")
            nc.tensor.matmul(
                lp_ps[:1, :1], ones_f[:, :1], lcol[:, :1], start=True, stop=True
            )
            nc.scalar.activation(
                out=loss_sb[:1, 0:1], in_=lp_ps[:1, :1], func=AF.Copy,
                scale=loss_scale,
            )
            nc.sync.dma_start(out=out_ext[:], in_=loss_sb[:1, 0:1])

    nc.compile()
    return nc


LAST_RESULT = None


def kernel(features, prototypes, logits):
    from concourse.bass_utils import run_bass_kernel_spmd

    global LAST_RESULT
    n_cores = 8
    B, D = features.shape
    K = prototypes.shape[0]
    B_loc = B // n_cores

    nc = build_nc(B_loc=B_loc, K=K, D=D, n_cores=n_cores)

    features = np.ascontiguousarray(features, dtype=np.float32)
    prototypes = np.ascontiguousarray(prototypes, dtype=np.float32)
    logits = np.ascontiguousarray(logits, dtype=np.float32)

    in_maps = [
        {
            "features": features[i * B_loc : (i + 1) * B_loc],
            "prototypes": prototypes,
            "logits": logits[i * B_loc : (i + 1) * B_loc],
        }
        for i in range(n_cores)
    ]
    res = run_bass_kernel_spmd(
        nc,
        in_maps,
        list(range(n_cores)),
        trace=bool(os.environ.get("CLIP_OT_TRACE")),
    )
    LAST_RESULT = res
    total = 0.0
    for i in range(n_cores):
        total += float(np.asarray(res.results[i]["out"], dtype=np.float64)[0])
    return np.float32(total)


# revision 39
# speedup vs baseline: 1.5088x; 1.1148x over previous
"""Trainium2 Bass kernel for nn_ClipOTLoss (CLIP-style OT/Sinkhorn loss).

Computes, for full inputs features[B,D], prototypes[K,D], logits[B,K]:
    w = normalize(prototypes, axis=1)
    sims = features @ w.T / TEMPERATURE
    soft_code = sinkhorn(sims)            (3 iters, eps=0.7)
    loss = -mean_b sum_k soft_code * log_softmax(logits)

Distribution: data-parallel over B across 8 NeuronCores; prototypes
replicated; the Sinkhorn row-marginal (sum over B per prototype k)
is a 16KB AllReduce per iteration.  Per-core partial losses are summed
on the host (no final AllReduce).

Structure (v2):
  - Sinkhorn preserves diagonal scaling: Q = E * A[k] * Bb[b] with
    E = exp(sims/eps); each iteration is one PE matvec (u-direction,
    partition-reduction over b) plus one DVE free-dim reduction
    (v-direction, against a GpSimd-broadcast A row).  No E^T is ever
    built.
  - Prologue keeps the Scalar engine on one activation table
    (Square/Sqrt) instead of thrashing Square/Ln/Exp per k-tile.
  - The iteration-0 u-matvec accumulates inside the main matmul phase,
    so the first AllReduce fires right after the last exp.
  - logits are streamed exactly once, during the AllReduce gaps:
    Scalar does exp+accum (for LSE), Vector writes EL = E*logits bf16
    into the slot vacated by w^T.  The epilogue is two DVE reduce
    sweeps:  s_b = sum_k E*A,  dotraw_b = sum_k EL*A,
    loss_b = LSE_b - dotraw_b/s_b.
"""

import os
import sys

import numpy as np

sys.path.insert(0, "/opt/trn_rl_repo")

import concourse.bass as bass  # noqa: E402
import concourse.bacc as bacc  # noqa: E402
import concourse.tile as tile  # noqa: E402
import concourse.mybir as mybir  # noqa: E402
from concourse.masks import make_identity  # noqa: E402

F32 = mybir.dt.float32
BF16 = mybir.dt.bfloat16
FP8 = mybir.dt.float8e4
AF = mybir.ActivationFunctionType
ALU = mybir.AluOpType

TEMPERATURE = 0.01
EPSILON = 0.7
NUM_ITERS = 3
TINY = 1e-8

P = 128  # partitions
NSLICE = 512  # max matmul free dim (one PSUM bank of f32)


def build_nc(B_loc=1024, K=4096, D=1024, n_cores=8):
    NB = B_loc // P  # number of 128-row b-blocks per core
    NK = K // P  # number of 128-wide k-chunks
    ND = D // P  # number of 128-deep d-chunks
    SF_F = 256.0  # feature pre-scale into fp8e4 range
    SF_W = 32.0  # prototype pre-scale into fp8e4 range
    exp_scale = 1.0 / (TEMPERATURE * EPSILON) / (SF_F * SF_W)
    r_marg = 1.0 / K
    c_marg = 1.0 / (B_loc * n_cores)
    loss_scale = 1.0 / (B_loc * n_cores)
    rg = [list(range(n_cores))]
    WARM_AR = bool(int(os.environ.get('CLIP_OT_WARM_AR', '1')))
    NWARM = 110  # PE clock warmers per AllReduce gap

    nc = bacc.Bacc(None, target_bir_lowering=False, debug=False)

    feats = nc.declare_dram_parameter("features", [B_loc, D], F32, isOutput=False)
    protos = nc.declare_dram_parameter("prototypes", [K, D], F32, isOutput=False)
    logits = nc.declare_dram_parameter("logits", [B_loc, K], F32, isOutput=False)
    out_ext = nc.declare_dram_parameter("out", [1], F32, isOutput=True)

    # collective bounce buffers (internal DRAM; outputs must be Shared)
    m_in_d = [nc.dram_tensor(f"cc_m_in{i}", [K], BF16) for i in range(NUM_ITERS)]
    m_out_d = [
        nc.dram_tensor(f"cc_m_out{i}", [K], BF16, addr_space="Shared")
        for i in range(NUM_ITERS)
    ]
    w_in_d = nc.dram_tensor("cc_w_in", [8], F32)
    w_out_d = nc.dram_tensor("cc_w_out", [8], F32, addr_space="Shared")

    with tile.TileContext(nc) as tc:
        with (
            tc.tile_pool(name="single", bufs=1) as single,
            tc.tile_pool(name="big", bufs=1) as bigp,
            tc.tile_pool(name="stage", bufs=6) as stage,
            tc.tile_pool(name="wsc", bufs=6) as wscp,
            tc.tile_pool(name="psmm", bufs=2, space="PSUM") as psmm,
            tc.tile_pool(name="psmv", bufs=2, space="PSUM") as psmv,
        ):
            # ---- packed small-tensor arenas ----
            smf = single.tile([P, 256 + 128 + NK * 3 + NB * 40], F32, tag="smf")
            smb = single.tile([P, 256 + NB + 8], BF16, tag="smb")

            class _Cols:
                def __init__(self, t):
                    self.t, self.off = t, 0

                def take(self, np_, nf):
                    ap = self.t[:np_, self.off : self.off + nf]
                    self.off += nf
                    return ap

            cf, cb = _Cols(smf), _Cols(smb)

            ident_f = cf.take(P, P)
            make_identity(nc, ident_f)
            ones_f = cf.take(P, 1)
            nc.vector.memset(ones_f, 1.0)
            norm2 = cf.take(P, NK)
            sqrtn = cf.take(P, NK)
            rn = cf.take(P, NK)
            cs_fl = cf.take(P, NB * 8)  # per-eighth colsum partials
            cs0 = cf.take(P, NB)
            Bb = cf.take(P, NB)
            bt = cf.take(P, NB)
            se_fl = cf.take(P, NB * 4)  # per-quarter sum(exp(logits))
            se_s = cf.take(P, NB)
            lse = cf.take(P, NB)
            s_col = cf.take(P, NB)
            dotraw = cf.take(P, NB)
            rs = cf.take(P, NB)
            dots = cf.take(P, NB)
            losses = cf.take(P, NB)
            lcol = cf.take(P, 1)
            loss_sb = cf.take(1, 8)
            warm_src = cf.take(P, 8)
            s4 = cf.take(P, NB * 4)  # tail quarter-partials (s)
            d4 = cf.take(P, NB * 4)  # tail quarter-partials (dot)
            v4 = cf.take(P, NB * 4)  # v-sweep quarter-partials

            ident_b = cb.take(P, P)
            make_identity(nc, ident_b)
            Bb_bf = cb.take(P, NB)
            ones_b1 = cb.take(1, P)
            nc.vector.memset(ones_b1, 1.0)

            # [32, 128]-layout iteration state: x[a, b] = x[k = a*128 + b]
            m32 = [single.tile([NK, P], BF16, tag=f"m32_{i}", name=f"m32_{i}") for i in range(NUM_ITERS)]
            A32 = single.tile([NK, P], F32, tag="A32")
            A32t = single.tile([NK, P], F32, tag="A32t")
            A32bf = single.tile([NK, P], BF16, tag="A32bf")
            at_flat = single.tile([1, K], BF16, tag="atflat")
            A_bc = single.tile([P, K], BF16, tag="Abc")
            msb = single.tile([1, K], BF16, tag="msb")  # m staging row (bf16 AR payload)
            warm_sb = single.tile([1, 8], F32, tag="warmsb")

            # ---- persistent big tensors ----
            E = bigp.tile([P, NB, K], BF16, tag="E")  # E[b,k], b-major
            wn_t = bigp.tile([P, ND, K], FP8, tag="W")  # w_norm^T [d,k], fp8
            f_t = bigp.tile([P, ND, B_loc], FP8, tag="F")  # features^T [d,b], fp8

            # =========================================================
            # Warm-up AllReduce: absorbs the CC-stream startup latency
            # so the first real AllReduce triggers immediately.
            # =========================================================
            if WARM_AR:
                nc.vector.memset(warm_sb[:], 0.0)
                nc.sync.dma_start(out=w_in_d[:], in_=warm_sb[:1, :])
                nc.gpsimd.collective_compute(
                    "AllReduce", ALU.add, replica_groups=rg,
                    ins=[w_in_d[:]], outs=[w_out_d[:]],
                )

            # =========================================================
            # Prologue A: prototypes -> normalized, bf16, transposed.
            # Scalar stays on the Sqrt table (Square lives in every
            # table) -- one ACT_TABLE_LOAD for the whole prologue.
            # =========================================================
            for kt in range(NK):
                wt = stage.tile([P, D], F32, tag="stage")
                nc.sync.dma_start(out=wt[:], in_=protos[kt * P : (kt + 1) * P, :])
                ws = wscp.tile([P, D], BF16, tag="wsc")
                nc.scalar.activation(
                    out=ws[:], in_=wt[:], func=AF.Square,
                    accum_out=norm2[:, kt : kt + 1],
                )
                # sqrt(norm2/SF_W^2) = ||w||/SF_W, so rn = SF_W/||w||
                nc.scalar.activation(
                    out=sqrtn[:, kt : kt + 1], in_=norm2[:, kt : kt + 1],
                    func=AF.Sqrt, scale=1.0 / (SF_W * SF_W),
                )
                nc.vector.reciprocal(out=rn[:, kt : kt + 1], in_=sqrtn[:, kt : kt + 1])
                # scale rows by 1/||w||, cast to bf16 (overwrites Square scratch)
                nc.vector.tensor_scalar(
                    out=ws[:], in0=wt[:], scalar1=rn[:, kt : kt + 1], scalar2=None,
                    op0=ALU.mult,
                )
                # transpose the tile's 8 d-chunks into one PSUM bank, then
                # one strided copy into the wn_t column
                tp = psmm.tile([P, ND * P], BF16, tag="mm")
                for j in range(ND):
                    nc.tensor.transpose(
                        tp[:, j * P : (j + 1) * P],
                        ws[:, j * P : (j + 1) * P],
                        ident_b[:],
                    )
                if kt % 2 == 0:
                    nc.scalar.copy(
                        out=wn_t[:, :, kt * P : (kt + 1) * P],
                        in_=tp[:].rearrange("p (j b) -> p j b", j=ND),
                    )
                else:
                    nc.vector.tensor_copy(
                        out=wn_t[:, :, kt * P : (kt + 1) * P],
                        in_=tp[:].rearrange("p (j b) -> p j b", j=ND),
                    )

            # =========================================================
            # Prologue B: features -> bf16 (DVE cast), transposed [d, b]
            # =========================================================
            for c in range(NB):
                ft_in = stage.tile([P, D], F32, tag="stage")
                nc.sync.dma_start(out=ft_in[:], in_=feats[c * P : (c + 1) * P, :])
                fb = wscp.tile([P, D], BF16, tag="wsc")
                nc.vector.tensor_scalar(
                    out=fb[:], in0=ft_in[:], scalar1=SF_F, scalar2=None,
                    op0=ALU.mult,
                )
                tp = psmm.tile([P, ND * P], BF16, tag="mm")
                for j in range(ND):
                    nc.tensor.transpose(
                        tp[:, j * P : (j + 1) * P],
                        fb[:, j * P : (j + 1) * P],
                        ident_b[:],
                    )
                if c % 2 == 0:
                    nc.scalar.copy(
                        out=f_t[:, :, c * P : (c + 1) * P],
                        in_=tp[:].rearrange("p (j b) -> p j b", j=ND),
                    )
                else:
                    nc.vector.tensor_copy(
                        out=f_t[:, :, c * P : (c + 1) * P],
                        in_=tp[:].rearrange("p (j b) -> p j b", j=ND),
                    )

            # =========================================================
            # Main matmul: sims_raw = f @ wn^T, E = exp(scale*sims_raw)
            # per b-block c, per 512-col k-slice: psum [128, 512].
            # The iteration-0 u-matvec (m0[k] = sum_b E[b,k]*Bb0[b])
            # accumulates into two [1, K/2] PSUM tiles as blocks finish,
            # so the first AllReduce fires right after the last exp.
            # =========================================================
            KQ4 = K // 4  # u-matvec accumulates in [1, 1024] PSUM quarters

            def mv_quarters(pfx):
                return [
                    psmv.tile([1, KQ4], F32, tag=f"mvq{q}", bufs=1, name=f"{pfx}_{q}")
                    for q in range(3)
                ]

            def emit_mv_fused(c, qt, first, last):
                # quarters 0-2, accumulated block-by-block
                for q in range(3):
                    for n in range(2):
                        o = q * KQ4 + n * NSLICE
                        nc.tensor.matmul(
                            qt[q][:1, n * NSLICE : (n + 1) * NSLICE],
                            Bb_bf[:, c : c + 1],
                            E[:, c, o : o + NSLICE],
                            start=first,
                            stop=last,
                        )

            def emit_mv_tail(it, qt, pfx):
                # copy quarters 0-2 to the staging row (parallel engines),
                # then quarter 3 reuses PSUM slot 0, then DMA + AR
                nc.scalar.copy(out=msb[:1, 0:KQ4], in_=qt[0][:1, :])
                nc.vector.tensor_copy(out=msb[:1, KQ4 : 2 * KQ4], in_=qt[1][:1, :])
                nc.scalar.copy(out=msb[:1, 2 * KQ4 : 3 * KQ4], in_=qt[2][:1, :])
                q3 = psmv.tile([1, KQ4], F32, tag="mvq0", bufs=1, name=f"{pfx}_3")
                for c in range(NB):
                    for n in range(2):
                        o = 3 * KQ4 + n * NSLICE
                        nc.tensor.matmul(
                            q3[:1, n * NSLICE : (n + 1) * NSLICE],
                            Bb_bf[:, c : c + 1],
                            E[:, c, o : o + NSLICE],
                            start=(c == 0),
                            stop=(c == NB - 1),
                        )
                nc.vector.tensor_copy(out=msb[:1, 3 * KQ4 :], in_=q3[:1, :])
                nc.sync.dma_start(out=m_in_d[it][:], in_=msb[:1, :])
                nc.gpsimd.collective_compute(
                    "AllReduce", ALU.add, replica_groups=rg,
                    ins=[m_in_d[it][:]], outs=[m_out_d[it][:]],
                )

            mv0 = mv_quarters("mv0")
            for c in range(NB):
                for e in range(K // NSLICE):
                    mm_ps = psmm.tile([P, NSLICE], F32, tag="mm")
                    for j in range(ND // 2):
                        nc.tensor.matmul(
                            mm_ps[:],
                            f_t[:, 2 * j : 2 * j + 2, c * P : (c + 1) * P],
                            wn_t[:, 2 * j : 2 * j + 2, e * NSLICE : (e + 1) * NSLICE],
                            start=(j == 0),
                            stop=(j == ND // 2 - 1),
                            perf_mode=mybir.MatmulPerfMode.DoubleRow,
                        )
                    nc.scalar.activation(
                        out=E[:, c, e * NSLICE : (e + 1) * NSLICE],
                        in_=mm_ps[:],
                        func=AF.Exp,
                        scale=exp_scale,
                        accum_out=cs_fl[:, c * 8 + e : c * 8 + e + 1],
                    )
                # Bb0 for block c: 1 / sum_k E[b,k]
                nc.vector.tensor_reduce(
                    out=cs0[:, c : c + 1],
                    in_=cs_fl[:, c * 8 : (c + 1) * 8].rearrange("p (a q) -> p a q", a=1),
                    axis=mybir.AxisListType.X,
                    op=ALU.add,
                )
                nc.vector.reciprocal(out=Bb[:, c : c + 1], in_=cs0[:, c : c + 1])
                nc.vector.tensor_copy(out=Bb_bf[:, c : c + 1], in_=Bb[:, c : c + 1])
                if c >= 1:
                    emit_mv_fused(c - 1, mv0, first=(c - 1 == 0), last=False)
            emit_mv_fused(NB - 1, mv0, first=False, last=True)
            emit_mv_tail(0, mv0, "mv0")

            # =========================================================
            # logits stream: LSE partials (Scalar) + EL = E*logits bf16
            # (Vector) into the slot vacated by wn_t.  Split across the
            # AllReduce gaps.  EL[b,k] reuses tag "W".
            # =========================================================
            KL = K // 4  # logits stream tile width
            EL = bigp.tile([P, NB, K], BF16, tag="W")
            vscr = bigp.tile([P, K], BF16, tag="F")  # v-sweep dump, aliases f_t

            def emit_logits(c):
                for q in range(4):
                    lt = stage.tile([P, KL], F32, tag="stage")
                    nc.sync.dma_start(
                        out=lt[:],
                        in_=logits[c * P : (c + 1) * P, q * KL : (q + 1) * KL],
                    )
                    sexp = wscp.tile([P, KL], BF16, tag="wsc")
                    nc.scalar.activation(
                        out=sexp[:], in_=lt[:], func=AF.Exp,
                        accum_out=se_fl[:, c * 4 + q : c * 4 + q + 1],
                    )
                    nc.vector.tensor_tensor(
                        out=EL[:, c, q * KL : (q + 1) * KL],
                        in0=E[:, c, q * KL : (q + 1) * KL],
                        in1=lt[:],
                        op=ALU.mult,
                    )

            def emit_warmers(n):
                # dependency-free PE busy-work: keeps the clock governor
                # at full p-state through an AllReduce gap
                wp = psmm.tile([P, P], BF16, tag="mm")
                for _ in range(n):
                    nc.tensor.transpose(wp[:, :], ident_b[:], ident_b[:])

            def emit_gated_warmers(it, n):
                # warmers that depend on the AllReduce result: they run
                # right before the u-matvec, bridging the A-update gap
                wp = psmm.tile([P, NK], BF16, tag="mm")
                for _ in range(n):
                    nc.tensor.transpose(wp[:, :NK], m32[it][:, :], ident_b[:NK, :NK])

            def emit_A_update(it):
                # m arrives as [32, 128]; A update stays in that layout.
                nc.sync.dma_start(
                    out=m32[it][:], in_=m_out_d[it][:].rearrange("(a b) -> a b", a=NK)
                )
                if it < NUM_ITERS - 1:
                    emit_gated_warmers(it, 40)
                if it == 0:
                    nc.vector.tensor_scalar(
                        out=A32t[:], in0=m32[it][:], scalar1=TINY, scalar2=None,
                        op0=ALU.add,
                    )
                    nc.vector.reciprocal(out=A32[:], in_=A32t[:])
                    nc.vector.tensor_scalar(
                        out=A32[:], in0=A32[:], scalar1=r_marg, scalar2=None,
                        op0=ALU.mult,
                    )
                else:
                    nc.vector.tensor_tensor(
                        out=A32t[:], in0=A32[:], in1=m32[it][:], op=ALU.mult
                    )
                    nc.vector.tensor_scalar(
                        out=A32t[:], in0=A32t[:], scalar1=TINY, scalar2=None,
                        op0=ALU.add,
                    )
                    nc.vector.reciprocal(out=A32t[:], in_=A32t[:])
                    nc.vector.tensor_tensor(
                        out=A32[:], in0=A32[:], in1=A32t[:], op=ALU.mult
                    )
                    nc.vector.tensor_scalar(
                        out=A32[:], in0=A32[:], scalar1=r_marg, scalar2=None,
                        op0=ALU.mult,
                    )
                nc.vector.tensor_copy(out=A32bf[:], in_=A32[:])
                nc.sync.dma_start(out=at_flat[:1, :], in_=A32bf[:])
                # broadcast the A row to all partitions on the (idle) PE:
                # ones[1,128]^T @ at_flat[1,512] -> [128,512] per slice
                for n in range(K // NSLICE):
                    bc_ps = psmm.tile([P, NSLICE], F32, tag="mm")
                    nc.tensor.matmul(
                        bc_ps[:],
                        ones_b1[:1, :],
                        at_flat[:1, n * NSLICE : (n + 1) * NSLICE],
                        start=True,
                        stop=True,
                    )
                    if n % 2 == 0:
                        nc.scalar.copy(
                            out=A_bc[:, n * NSLICE : (n + 1) * NSLICE], in_=bc_ps[:]
                        )
                    else:
                        nc.vector.tensor_copy(
                            out=A_bc[:, n * NSLICE : (n + 1) * NSLICE], in_=bc_ps[:]
                        )

            # =========================================================
            # Sinkhorn iterations
            # =========================================================
            for it in range(NUM_ITERS):
                if it == 0:
                    emit_warmers(NWARM)
                    for c in range(3):
                        emit_logits(c)
                emit_A_update(it)
                if it < NUM_ITERS - 1:
                    # v(c) -> Bb(c) -> u(c), pipelined per block.  Blocks
                    # 0-1 use one DVE STT each (accum = Bb*v_raw); blocks
                    # 2-7 use DVE TT quarters reduced on the Scalar engine
                    # (engines balanced, u starts as soon as block 0 lands).
                    mv = mv_quarters(f"mv{it + 1}")

                    def finish_bb(c, folded):
                        if not folded:
                            # bt currently holds v_raw; fold Bb in
                            nc.vector.tensor_tensor(
                                out=bt[:, c : c + 1], in0=Bb[:, c : c + 1],
                                in1=bt[:, c : c + 1], op=ALU.mult,
                            )
                        nc.vector.tensor_scalar(
                            out=bt[:, c : c + 1], in0=bt[:, c : c + 1],
                            scalar1=TINY, scalar2=None, op0=ALU.add,
                        )
                        nc.vector.reciprocal(out=bt[:, c : c + 1], in_=bt[:, c : c + 1])
                        nc.vector.tensor_tensor(
                            out=Bb[:, c : c + 1], in0=Bb[:, c : c + 1],
                            in1=bt[:, c : c + 1], op=ALU.mult,
                        )
                        nc.vector.tensor_scalar(
                            out=Bb[:, c : c + 1], in0=Bb[:, c : c + 1],
                            scalar1=c_marg, scalar2=None, op0=ALU.mult,
                        )
                        nc.vector.tensor_copy(
                            out=Bb_bf[:, c : c + 1], in_=Bb[:, c : c + 1]
                        )
                        emit_mv_fused(c, mv, first=(c == 0), last=(c == NB - 1))

                    for c in range(2):
                        # accum = sum_k (E*Bb)*A = Bb * v_raw, in one op
                        nc.vector.scalar_tensor_tensor(
                            out=vscr[:],
                            in0=E[:, c, :],
                            scalar=Bb[:, c : c + 1],
                            in1=A_bc[:],
                            op0=ALU.mult,
                            op1=ALU.mult,
                            accum_out=bt[:, c : c + 1],
                        )
                        finish_bb(c, folded=True)
                    def reduce_v4(c):
                        nc.vector.tensor_reduce(
                            out=bt[:, c : c + 1],
                            in_=v4[:, c * 4 : (c + 1) * 4].rearrange(
                                "p (a q) -> p a q", a=1
                            ),
                            axis=mybir.AxisListType.X,
                            op=ALU.add,
                        )

                    # stagger: emit TTs of block c, then the (Scalar-fed)
                    # reduce of block c-1, so the DVE never waits on Scalar
                    for c in range(2, NB):
                        for q in range(4):
                            vq = wscp.tile([P, KQ4], BF16, tag="wsc")
                            nc.vector.tensor_tensor(
                                out=vq[:],
                                in0=E[:, c, q * KQ4 : (q + 1) * KQ4],
                                in1=A_bc[:, q * KQ4 : (q + 1) * KQ4],
                                op=ALU.mult,
                            )
                            nc.scalar.activation(
                                out=vq[:], in_=vq[:], func=AF.Copy,
                                accum_out=v4[:, c * 4 + q : c * 4 + q + 1],
                            )
                        if c > 2:
                            reduce_v4(c - 1)
                            finish_bb(c - 1, folded=False)
                    reduce_v4(NB - 1)
                    finish_bb(NB - 1, folded=False)
                    emit_mv_tail(it + 1, mv, f"mv{it + 1}")
                    if it == 0:
                        emit_warmers(NWARM)
                        for c in range(3, 6):
                            emit_logits(c)
                    else:
                        for c in range(6, NB):
                            emit_logits(c)

            # =========================================================
            # Final: s_b = sum_k E*A, dotraw_b = sum_k EL*A,
            # loss_b = LSE_b - dotraw_b / s_b
            # =========================================================
            # blocks 2-7: DVE TT quarters reduced on Scalar (runs both
            # engines); blocks 0-1: DVE STTs at the end (Scalar drains)
            for c in range(2, NB):
                for src, part in ((E, s4), (EL, d4)):
                    for q in range(4):
                        tq = wscp.tile([P, KQ4], BF16, tag="wsc")
                        nc.vector.tensor_tensor(
                            out=tq[:],
                            in0=src[:, c, q * KQ4 : (q + 1) * KQ4],
                            in1=A_bc[:, q * KQ4 : (q + 1) * KQ4],
                            op=ALU.mult,
                        )
                        nc.scalar.activation(
                            out=tq[:], in_=tq[:], func=AF.Copy,
                            accum_out=part[:, c * 4 + q : c * 4 + q + 1],
                        )
            for c in range(2):
                nc.vector.scalar_tensor_tensor(
                    out=vscr[:], in0=E[:, c, :], scalar=1.0, in1=A_bc[:],
                    op0=ALU.mult, op1=ALU.mult,
                    accum_out=s_col[:, c : c + 1],
                )
                nc.vector.scalar_tensor_tensor(
                    out=vscr[:], in0=EL[:, c, :], scalar=1.0, in1=A_bc[:],
                    op0=ALU.mult, op1=ALU.mult,
                    accum_out=dotraw[:, c : c + 1],
                )
            for c in range(2, NB):
                for part, dst in ((s4, s_col), (d4, dotraw)):
                    nc.vector.tensor_reduce(
                        out=dst[:, c : c + 1],
                        in_=part[:, c * 4 : (c + 1) * 4].rearrange(
                            "p (a q) -> p a q", a=1
                        ),
                        axis=mybir.AxisListType.X,
                        op=ALU.add,
                    )
            se_q = se_fl.rearrange("p (c q) -> p c q", q=4)
            nc.vector.tensor_reduce(
                out=se_s, in_=se_q, axis=mybir.AxisListType.X, op=ALU.add
            )
            nc.scalar.activation(out=lse, in_=se_s, func=AF.Ln)
            nc.vector.reciprocal(out=rs[:], in_=s_col[:])
            nc.vector.tensor_tensor(out=dots, in0=dotraw, in1=rs, op=ALU.mult)
            nc.vector.tensor_tensor(out=losses, in0=lse, in1=dots, op=ALU.subtract)
            nc.vector.tensor_reduce(
                out=lcol, in_=losses, axis=mybir.AxisListType.X, op=ALU.add
            )
            lp_ps = psmm.tile([1, NSLICE], F32, tag="mm")
            nc.tensor.matmul(
                lp_ps[:1, :1], ones_f[:, :1], lcol[:, :1], start=True, stop=True
            )
            nc.scalar.activation(
                out=loss_sb[:1, 0:1], in_=lp_ps[:1, :1], func=AF.Copy,
                scale=loss_scale,
            )
            nc.sync.dma_start(out=out_ext[:], in_=loss_sb[:1, 0:1])

    nc.compile()
    return nc


LAST_RESULT = None


def kernel(features, prototypes, logits):
    from concourse.bass_utils import run_bass_kernel_spmd

    global LAST_RESULT
    n_cores = 8
    B, D = features.shape
    K = prototypes.shape[0]
    B_loc = B // n_cores

    nc = build_nc(B_loc=B_loc, K=K, D=D, n_cores=n_cores)

    features = np.ascontiguousarray(features, dtype=np.float32)
    prototypes = np.ascontiguousarray(prototypes, dtype=np.float32)
    logits = np.ascontiguousarray(logits, dtype=np.float32)

    in_maps = [
        {
            "features": features[i * B_loc : (i + 1) * B_loc],
            "prototypes": prototypes,
            "logits": logits[i * B_loc : (i + 1) * B_loc],
        }
        for i in range(n_cores)
    ]
    res = run_bass_kernel_spmd(
        nc,
        in_maps,
        list(range(n_cores)),
        trace=bool(os.environ.get("CLIP_OT_TRACE")),
    )
    LAST_RESULT = res
    total = 0.0
    for i in range(n_cores):
        total += float(np.asarray(res.results[i]["out"], dtype=np.float64)[0])
    return np.float32(total)


# revision 40
# speedup vs baseline: 1.5432x; 1.0228x over previous
"""Trainium2 Bass kernel for nn_ClipOTLoss (CLIP-style OT/Sinkhorn loss).

Computes, for full inputs features[B,D], prototypes[K,D], logits[B,K]:
    w = normalize(prototypes, axis=1)
    sims = features @ w.T / TEMPERATURE
    soft_code = sinkhorn(sims)            (3 iters, eps=0.7)
    loss = -mean_b sum_k soft_code * log_softmax(logits)

Distribution: data-parallel over B across 8 NeuronCores; prototypes
replicated; the Sinkhorn row-marginal (sum over B per prototype k)
is a 16KB AllReduce per iteration.  Per-core partial losses are summed
on the host (no final AllReduce).

Structure (v2):
  - Sinkhorn preserves diagonal scaling: Q = E * A[k] * Bb[b] with
    E = exp(sims/eps); each iteration is one PE matvec (u-direction,
    partition-reduction over b) plus one DVE free-dim reduction
    (v-direction, against a GpSimd-broadcast A row).  No E^T is ever
    built.
  - Prologue keeps the Scalar engine on one activation table
    (Square/Sqrt) instead of thrashing Square/Ln/Exp per k-tile.
  - The iteration-0 u-matvec accumulates inside the main matmul phase,
    so the first AllReduce fires right after the last exp.
  - logits are streamed exactly once, during the AllReduce gaps:
    Scalar does exp+accum (for LSE), Vector writes EL = E*logits bf16
    into the slot vacated by w^T.  The epilogue is two DVE reduce
    sweeps:  s_b = sum_k E*A,  dotraw_b = sum_k EL*A,
    loss_b = LSE_b - dotraw_b/s_b.
"""

import os
import sys

import numpy as np

sys.path.insert(0, "/opt/trn_rl_repo")

import concourse.bass as bass  # noqa: E402
import concourse.bacc as bacc  # noqa: E402
import concourse.tile as tile  # noqa: E402
import concourse.mybir as mybir  # noqa: E402
from concourse.masks import make_identity  # noqa: E402

F32 = mybir.dt.float32
BF16 = mybir.dt.bfloat16
FP8 = mybir.dt.float8e4
AF = mybir.ActivationFunctionType
ALU = mybir.AluOpType

TEMPERATURE = 0.01
EPSILON = 0.7
NUM_ITERS = 3
TINY = 1e-8

P = 128  # partitions
NSLICE = 512  # max matmul free dim (one PSUM bank of f32)


def build_nc(B_loc=1024, K=4096, D=1024, n_cores=8):
    NB = B_loc // P  # number of 128-row b-blocks per core
    NK = K // P  # number of 128-wide k-chunks
    ND = D // P  # number of 128-deep d-chunks
    SF_F = 256.0  # feature pre-scale into fp8e4 range
    SF_W = 32.0  # prototype pre-scale into fp8e4 range
    exp_scale = 1.0 / (TEMPERATURE * EPSILON) / (SF_F * SF_W)
    r_marg = 1.0 / K
    c_marg = 1.0 / (B_loc * n_cores)
    loss_scale = 1.0 / (B_loc * n_cores)
    rg = [list(range(n_cores))]
    WARM_AR = bool(int(os.environ.get('CLIP_OT_WARM_AR', '1')))
    NWARM = 110  # PE clock warmers per AllReduce gap

    nc = bacc.Bacc(None, target_bir_lowering=False, debug=False)

    feats = nc.declare_dram_parameter("features", [B_loc, D], F32, isOutput=False)
    protos = nc.declare_dram_parameter("prototypes", [K, D], F32, isOutput=False)
    logits = nc.declare_dram_parameter("logits", [B_loc, K], F32, isOutput=False)
    out_ext = nc.declare_dram_parameter("out", [1], F32, isOutput=True)

    # collective bounce buffers (internal DRAM; outputs must be Shared)
    m_in_d = [nc.dram_tensor(f"cc_m_in{i}", [K], BF16) for i in range(NUM_ITERS)]
    m_out_d = [
        nc.dram_tensor(f"cc_m_out{i}", [K], BF16, addr_space="Shared")
        for i in range(NUM_ITERS)
    ]
    w_in_d = nc.dram_tensor("cc_w_in", [8], F32)
    w_out_d = nc.dram_tensor("cc_w_out", [8], F32, addr_space="Shared")

    with tile.TileContext(nc) as tc:
        with (
            tc.tile_pool(name="single", bufs=1) as single,
            tc.tile_pool(name="big", bufs=1) as bigp,
            tc.tile_pool(name="stage", bufs=6) as stage,
            tc.tile_pool(name="wsc", bufs=6) as wscp,
            tc.tile_pool(name="psmm", bufs=2, space="PSUM") as psmm,
            tc.tile_pool(name="psmv", bufs=2, space="PSUM") as psmv,
        ):
            # ---- packed small-tensor arenas ----
            smf = single.tile([P, 256 + 128 + NK * 3 + NB * 40], F32, tag="smf")
            smb = single.tile([P, 256 + NB + 8], BF16, tag="smb")

            class _Cols:
                def __init__(self, t):
                    self.t, self.off = t, 0

                def take(self, np_, nf):
                    ap = self.t[:np_, self.off : self.off + nf]
                    self.off += nf
                    return ap

            cf, cb = _Cols(smf), _Cols(smb)

            ident_f = cf.take(P, P)
            make_identity(nc, ident_f)
            ones_f = cf.take(P, 1)
            nc.vector.memset(ones_f, 1.0)
            norm2 = cf.take(P, NK)
            sqrtn = cf.take(P, NK)
            rn = cf.take(P, NK)
            cs_fl = cf.take(P, NB * 8)  # per-eighth colsum partials
            cs0 = cf.take(P, NB)
            Bb = cf.take(P, NB)
            bt = cf.take(P, NB)
            se_fl = cf.take(P, NB * 4)  # per-quarter sum(exp(logits))
            se_s = cf.take(P, NB)
            lse = cf.take(P, NB)
            s_col = cf.take(P, NB)
            dotraw = cf.take(P, NB)
            rs = cf.take(P, NB)
            dots = cf.take(P, NB)
            losses = cf.take(P, NB)
            lcol = cf.take(P, 1)
            loss_sb = cf.take(1, 8)
            warm_src = cf.take(P, 8)
            s4 = cf.take(P, NB * 4)  # tail quarter-partials (s)
            d4 = cf.take(P, NB * 4)  # tail quarter-partials (dot)
            v4 = cf.take(P, NB * 4)  # v-sweep quarter-partials

            ident_b = cb.take(P, P)
            make_identity(nc, ident_b)
            Bb_bf = cb.take(P, NB)
            ones_b1 = cb.take(1, P)
            nc.vector.memset(ones_b1, 1.0)

            # [32, 128]-layout iteration state: x[a, b] = x[k = a*128 + b]
            m32 = [single.tile([NK, P], BF16, tag=f"m32_{i}", name=f"m32_{i}") for i in range(NUM_ITERS)]
            A32 = single.tile([NK, P], F32, tag="A32")
            A32t = single.tile([NK, P], F32, tag="A32t")
            A32bf = single.tile([NK, P], BF16, tag="A32bf")
            at_flat = single.tile([1, K], BF16, tag="atflat")
            A_bc = single.tile([P, K], BF16, tag="Abc")
            msb = single.tile([1, K], BF16, tag="msb")  # m staging row (bf16 AR payload)
            warm_sb = single.tile([1, 8], F32, tag="warmsb")

            # ---- persistent big tensors ----
            E = bigp.tile([P, NB, K], BF16, tag="E")  # E[b,k], b-major
            wn_t = bigp.tile([P, ND, K], FP8, tag="W")  # w_norm^T [d,k], fp8
            f_t = bigp.tile([P, ND, B_loc], FP8, tag="F")  # features^T [d,b], fp8

            # =========================================================
            # Warm-up AllReduce: absorbs the CC-stream startup latency
            # so the first real AllReduce triggers immediately.
            # =========================================================
            if WARM_AR:
                nc.vector.memset(warm_sb[:], 0.0)
                nc.sync.dma_start(out=w_in_d[:], in_=warm_sb[:1, :])
                nc.gpsimd.collective_compute(
                    "AllReduce", ALU.add, replica_groups=rg,
                    ins=[w_in_d[:]], outs=[w_out_d[:]],
                )

            # =========================================================
            # Prologue A: prototypes -> normalized, bf16, transposed.
            # Scalar stays on the Sqrt table (Square lives in every
            # table) -- one ACT_TABLE_LOAD for the whole prologue.
            # =========================================================
            for kt in range(NK):
                wt = stage.tile([P, D], F32, tag="stage")
                nc.sync.dma_start(out=wt[:], in_=protos[kt * P : (kt + 1) * P, :])
                ws = wscp.tile([P, D], BF16, tag="wsc")
                nc.scalar.activation(
                    out=ws[:], in_=wt[:], func=AF.Square,
                    accum_out=norm2[:, kt : kt + 1],
                )
                # sqrt(norm2/SF_W^2) = ||w||/SF_W, so rn = SF_W/||w||
                nc.scalar.activation(
                    out=sqrtn[:, kt : kt + 1], in_=norm2[:, kt : kt + 1],
                    func=AF.Sqrt, scale=1.0 / (SF_W * SF_W),
                )
                nc.vector.reciprocal(out=rn[:, kt : kt + 1], in_=sqrtn[:, kt : kt + 1])
                # scale rows by 1/||w||, cast to bf16 (overwrites Square scratch)
                nc.vector.tensor_scalar(
                    out=ws[:], in0=wt[:], scalar1=rn[:, kt : kt + 1], scalar2=None,
                    op0=ALU.mult,
                )
                # transpose the tile's 8 d-chunks into one PSUM bank, then
                # one strided copy into the wn_t column
                tp = psmm.tile([P, ND * P], BF16, tag="mm")
                for j in range(ND):
                    nc.tensor.transpose(
                        tp[:, j * P : (j + 1) * P],
                        ws[:, j * P : (j + 1) * P],
                        ident_b[:],
                    )
                if kt % 2 == 0:
                    nc.scalar.copy(
                        out=wn_t[:, :, kt * P : (kt + 1) * P],
                        in_=tp[:].rearrange("p (j b) -> p j b", j=ND),
                    )
                else:
                    nc.vector.tensor_copy(
                        out=wn_t[:, :, kt * P : (kt + 1) * P],
                        in_=tp[:].rearrange("p (j b) -> p j b", j=ND),
                    )

            # =========================================================
            # Prologue B: features -> bf16 (DVE cast), transposed [d, b]
            # =========================================================
            for c in range(NB):
                ft_in = stage.tile([P, D], F32, tag="stage")
                nc.sync.dma_start(out=ft_in[:], in_=feats[c * P : (c + 1) * P, :])
                fb = wscp.tile([P, D], BF16, tag="wsc")
                nc.vector.tensor_scalar(
                    out=fb[:], in0=ft_in[:], scalar1=SF_F, scalar2=None,
                    op0=ALU.mult,
                )
                tp = psmm.tile([P, ND * P], BF16, tag="mm")
                for j in range(ND):
                    nc.tensor.transpose(
                        tp[:, j * P : (j + 1) * P],
                        fb[:, j * P : (j + 1) * P],
                        ident_b[:],
                    )
                if c % 2 == 0:
                    nc.scalar.copy(
                        out=f_t[:, :, c * P : (c + 1) * P],
                        in_=tp[:].rearrange("p (j b) -> p j b", j=ND),
                    )
                else:
                    nc.vector.tensor_copy(
                        out=f_t[:, :, c * P : (c + 1) * P],
                        in_=tp[:].rearrange("p (j b) -> p j b", j=ND),
                    )

            # =========================================================
            # Main matmul: sims_raw = f @ wn^T, E = exp(scale*sims_raw)
            # per b-block c, per 512-col k-slice: psum [128, 512].
            # The iteration-0 u-matvec (m0[k] = sum_b E[b,k]*Bb0[b])
            # accumulates into two [1, K/2] PSUM tiles as blocks finish,
            # so the first AllReduce fires right after the last exp.
            # =========================================================
            KQ4 = K // 4  # u-matvec accumulates in [1, 1024] PSUM quarters

            def mv_quarters(pfx):
                return [
                    psmv.tile([1, KQ4], F32, tag=f"mvq{q}", bufs=1, name=f"{pfx}_{q}")
                    for q in range(3)
                ]

            def emit_mv_fused(c, qt, first, last):
                # quarters 0-2, accumulated block-by-block
                for q in range(3):
                    for n in range(2):
                        o = q * KQ4 + n * NSLICE
                        nc.tensor.matmul(
                            qt[q][:1, n * NSLICE : (n + 1) * NSLICE],
                            Bb_bf[:, c : c + 1],
                            E[:, c, o : o + NSLICE],
                            start=first,
                            stop=last,
                        )

            def emit_mv_tail(it, qt, pfx):
                # copy quarters 0-2 to the staging row (parallel engines),
                # then quarter 3 reuses PSUM slot 0, then DMA + AR
                nc.scalar.copy(out=msb[:1, 0:KQ4], in_=qt[0][:1, :])
                nc.vector.tensor_copy(out=msb[:1, KQ4 : 2 * KQ4], in_=qt[1][:1, :])
                nc.scalar.copy(out=msb[:1, 2 * KQ4 : 3 * KQ4], in_=qt[2][:1, :])
                q3 = psmv.tile([1, KQ4], F32, tag="mvq0", bufs=1, name=f"{pfx}_3")
                for c in range(NB):
                    for n in range(2):
                        o = 3 * KQ4 + n * NSLICE
                        nc.tensor.matmul(
                            q3[:1, n * NSLICE : (n + 1) * NSLICE],
                            Bb_bf[:, c : c + 1],
                            E[:, c, o : o + NSLICE],
                            start=(c == 0),
                            stop=(c == NB - 1),
                        )
                nc.vector.tensor_copy(out=msb[:1, 3 * KQ4 :], in_=q3[:1, :])
                nc.sync.dma_start(out=m_in_d[it][:], in_=msb[:1, :])
                nc.gpsimd.collective_compute(
                    "AllReduce", ALU.add, replica_groups=rg,
                    ins=[m_in_d[it][:]], outs=[m_out_d[it][:]],
                )

            KL = K // 4  # logits stream tile width

            def emit_lse(c, q):
                lt = stage.tile([P, KL], F32, tag="stage")
                nc.sync.dma_start(
                    out=lt[:],
                    in_=logits[c * P : (c + 1) * P, q * KL : (q + 1) * KL],
                )
                sexp = wscp.tile([P, KL], BF16, tag="wsc")
                nc.scalar.activation(
                    out=sexp[:], in_=lt[:], func=AF.Exp,
                    accum_out=se_fl[:, c * 4 + q : c * 4 + q + 1],
                )

            mv0 = mv_quarters("mv0")
            for c in range(NB):
                for e in range(K // NSLICE):
                    mm_ps = psmm.tile([P, NSLICE], F32, tag="mm")
                    for j in range(ND // 2):
                        nc.tensor.matmul(
                            mm_ps[:],
                            f_t[:, 2 * j : 2 * j + 2, c * P : (c + 1) * P],
                            wn_t[:, 2 * j : 2 * j + 2, e * NSLICE : (e + 1) * NSLICE],
                            start=(j == 0),
                            stop=(j == ND // 2 - 1),
                            perf_mode=mybir.MatmulPerfMode.DoubleRow,
                        )
                    nc.scalar.activation(
                        out=E[:, c, e * NSLICE : (e + 1) * NSLICE],
                        in_=mm_ps[:],
                        func=AF.Exp,
                        scale=exp_scale,
                        accum_out=cs_fl[:, c * 8 + e : c * 8 + e + 1],
                    )
                    if e % 2 == 1:
                        emit_lse(c, e // 2)
                # Bb0 for block c: 1 / sum_k E[b,k]
                nc.vector.tensor_reduce(
                    out=cs0[:, c : c + 1],
                    in_=cs_fl[:, c * 8 : (c + 1) * 8].rearrange("p (a q) -> p a q", a=1),
                    axis=mybir.AxisListType.X,
                    op=ALU.add,
                )
                nc.vector.reciprocal(out=Bb[:, c : c + 1], in_=cs0[:, c : c + 1])
                nc.vector.tensor_copy(out=Bb_bf[:, c : c + 1], in_=Bb[:, c : c + 1])
                if c >= 1:
                    emit_mv_fused(c - 1, mv0, first=(c - 1 == 0), last=False)
            emit_mv_fused(NB - 1, mv0, first=False, last=True)
            emit_mv_tail(0, mv0, "mv0")

            # =========================================================
            # logits stream: LSE partials (Scalar) + EL = E*logits bf16
            # (Vector) into the slot vacated by wn_t.  Split across the
            # AllReduce gaps.  EL[b,k] reuses tag "W".
            # =========================================================
            EL = bigp.tile([P, NB, K], BF16, tag="W")
            vscr = bigp.tile([P, K], BF16, tag="F")  # v-sweep dump, aliases f_t

            def emit_el(c):
                # second logits pass: EL = E*logits (DVE only; logits
                # re-DMA'd while the HBM link is otherwise idle)
                for q in range(4):
                    lt = stage.tile([P, KL], F32, tag="stage")
                    nc.sync.dma_start(
                        out=lt[:],
                        in_=logits[c * P : (c + 1) * P, q * KL : (q + 1) * KL],
                    )
                    nc.vector.tensor_tensor(
                        out=EL[:, c, q * KL : (q + 1) * KL],
                        in0=E[:, c, q * KL : (q + 1) * KL],
                        in1=lt[:],
                        op=ALU.mult,
                    )

            def emit_warmers(n):
                # dependency-free PE busy-work: keeps the clock governor
                # at full p-state through an AllReduce gap
                wp = psmm.tile([P, P], BF16, tag="mm")
                for _ in range(n):
                    nc.tensor.transpose(wp[:, :], ident_b[:], ident_b[:])

            def emit_gated_warmers(it, n):
                # warmers that depend on the AllReduce result: they run
                # right before the u-matvec, bridging the A-update gap
                wp = psmm.tile([P, NK], BF16, tag="mm")
                for _ in range(n):
                    nc.tensor.transpose(wp[:, :NK], m32[it][:, :], ident_b[:NK, :NK])

            def emit_A_update(it):
                # m arrives as [32, 128]; A update stays in that layout.
                nc.sync.dma_start(
                    out=m32[it][:], in_=m_out_d[it][:].rearrange("(a b) -> a b", a=NK)
                )
                if it < NUM_ITERS - 1:
                    emit_gated_warmers(it, 40)
                if it == 0:
                    nc.vector.tensor_scalar(
                        out=A32t[:], in0=m32[it][:], scalar1=TINY, scalar2=None,
                        op0=ALU.add,
                    )
                    nc.vector.reciprocal(out=A32[:], in_=A32t[:])
                    nc.vector.tensor_scalar(
                        out=A32[:], in0=A32[:], scalar1=r_marg, scalar2=None,
                        op0=ALU.mult,
                    )
                else:
                    nc.vector.tensor_tensor(
                        out=A32t[:], in0=A32[:], in1=m32[it][:], op=ALU.mult
                    )
                    nc.vector.tensor_scalar(
                        out=A32t[:], in0=A32t[:], scalar1=TINY, scalar2=None,
                        op0=ALU.add,
                    )
                    nc.vector.reciprocal(out=A32t[:], in_=A32t[:])
                    nc.vector.tensor_tensor(
                        out=A32[:], in0=A32[:], in1=A32t[:], op=ALU.mult
                    )
                    nc.vector.tensor_scalar(
                        out=A32[:], in0=A32[:], scalar1=r_marg, scalar2=None,
                        op0=ALU.mult,
                    )
                nc.vector.tensor_copy(out=A32bf[:], in_=A32[:])
                nc.sync.dma_start(out=at_flat[:1, :], in_=A32bf[:])
                # broadcast the A row to all partitions on the (idle) PE:
                # ones[1,128]^T @ at_flat[1,512] -> [128,512] per slice
                for n in range(K // NSLICE):
                    bc_ps = psmm.tile([P, NSLICE], F32, tag="mm")
                    nc.tensor.matmul(
                        bc_ps[:],
                        ones_b1[:1, :],
                        at_flat[:1, n * NSLICE : (n + 1) * NSLICE],
                        start=True,
                        stop=True,
                    )
                    if n % 2 == 0:
                        nc.scalar.copy(
                            out=A_bc[:, n * NSLICE : (n + 1) * NSLICE], in_=bc_ps[:]
                        )
                    else:
                        nc.vector.tensor_copy(
                            out=A_bc[:, n * NSLICE : (n + 1) * NSLICE], in_=bc_ps[:]
                        )

            # =========================================================
            # Sinkhorn iterations
            # =========================================================
            for it in range(NUM_ITERS):
                if it == 0:
                    emit_warmers(NWARM)
                    for c in range(3):
                        emit_el(c)
                emit_A_update(it)
                if it < NUM_ITERS - 1:
                    # v(c) -> Bb(c) -> u(c), pipelined per block.  Blocks
                    # 0-1 use one DVE STT each (accum = Bb*v_raw); blocks
                    # 2-7 use DVE TT quarters reduced on the Scalar engine
                    # (engines balanced, u starts as soon as block 0 lands).
                    mv = mv_quarters(f"mv{it + 1}")

                    def finish_bb(c, folded):
                        if not folded:
                            # bt currently holds v_raw; fold Bb in
                            nc.vector.tensor_tensor(
                                out=bt[:, c : c + 1], in0=Bb[:, c : c + 1],
                                in1=bt[:, c : c + 1], op=ALU.mult,
                            )
                        nc.vector.tensor_scalar(
                            out=bt[:, c : c + 1], in0=bt[:, c : c + 1],
                            scalar1=TINY, scalar2=None, op0=ALU.add,
                        )
                        nc.vector.reciprocal(out=bt[:, c : c + 1], in_=bt[:, c : c + 1])
                        nc.vector.tensor_tensor(
                            out=Bb[:, c : c + 1], in0=Bb[:, c : c + 1],
                            in1=bt[:, c : c + 1], op=ALU.mult,
                        )
                        nc.vector.tensor_scalar(
                            out=Bb[:, c : c + 1], in0=Bb[:, c : c + 1],
                            scalar1=c_marg, scalar2=None, op0=ALU.mult,
                        )
                        nc.vector.tensor_copy(
                            out=Bb_bf[:, c : c + 1], in_=Bb[:, c : c + 1]
                        )
                        emit_mv_fused(c, mv, first=(c == 0), last=(c == NB - 1))

                    for c in range(3):
                        # accum = sum_k (E*Bb)*A = Bb * v_raw, in one op
                        nc.vector.scalar_tensor_tensor(
                            out=vscr[:],
                            in0=E[:, c, :],
                            scalar=Bb[:, c : c + 1],
                            in1=A_bc[:],
                            op0=ALU.mult,
                            op1=ALU.mult,
                            accum_out=bt[:, c : c + 1],
                        )
                        finish_bb(c, folded=True)
                    def reduce_v4(c):
                        nc.vector.tensor_reduce(
                            out=bt[:, c : c + 1],
                            in_=v4[:, c * 4 : (c + 1) * 4].rearrange(
                                "p (a q) -> p a q", a=1
                            ),
                            axis=mybir.AxisListType.X,
                            op=ALU.add,
                        )

                    # stagger: emit TTs of block c, then the (Scalar-fed)
                    # reduce of block c-1, so the DVE never waits on Scalar
                    for c in range(3, NB):
                        for q in range(4):
                            vq = wscp.tile([P, KQ4], BF16, tag="wsc")
                            nc.vector.tensor_tensor(
                                out=vq[:],
                                in0=E[:, c, q * KQ4 : (q + 1) * KQ4],
                                in1=A_bc[:, q * KQ4 : (q + 1) * KQ4],
                                op=ALU.mult,
                            )
                            nc.scalar.activation(
                                out=vq[:], in_=vq[:], func=AF.Copy,
                                accum_out=v4[:, c * 4 + q : c * 4 + q + 1],
                            )
                        if c > 3:
                            reduce_v4(c - 1)
                            finish_bb(c - 1, folded=False)
                    reduce_v4(NB - 1)
                    finish_bb(NB - 1, folded=False)
                    emit_mv_tail(it + 1, mv, f"mv{it + 1}")
                    if it == 0:
                        emit_warmers(NWARM)
                        for c in range(3, 6):
                            emit_el(c)
                    else:
                        for c in range(6, NB):
                            emit_el(c)

            # =========================================================
            # Final: s_b = sum_k E*A, dotraw_b = sum_k EL*A,
            # loss_b = LSE_b - dotraw_b / s_b
            # =========================================================
            # blocks 2-7: DVE TT quarters reduced on Scalar (runs both
            # engines); blocks 0-1: DVE STTs at the end (Scalar drains)
            for c in range(3, NB):
                for src, part in ((E, s4), (EL, d4)):
                    for q in range(4):
                        tq = wscp.tile([P, KQ4], BF16, tag="wsc")
                        nc.vector.tensor_tensor(
                            out=tq[:],
                            in0=src[:, c, q * KQ4 : (q + 1) * KQ4],
                            in1=A_bc[:, q * KQ4 : (q + 1) * KQ4],
                            op=ALU.mult,
                        )
                        nc.scalar.activation(
                            out=tq[:], in_=tq[:], func=AF.Copy,
                            accum_out=part[:, c * 4 + q : c * 4 + q + 1],
                        )
            for c in range(3):
                nc.vector.scalar_tensor_tensor(
                    out=vscr[:], in0=E[:, c, :], scalar=1.0, in1=A_bc[:],
                    op0=ALU.mult, op1=ALU.mult,
                    accum_out=s_col[:, c : c + 1],
                )
                nc.vector.scalar_tensor_tensor(
                    out=vscr[:], in0=EL[:, c, :], scalar=1.0, in1=A_bc[:],
                    op0=ALU.mult, op1=ALU.mult,
                    accum_out=dotraw[:, c : c + 1],
                )
            for c in range(3, NB):
                for part, dst in ((s4, s_col), (d4, dotraw)):
                    nc.vector.tensor_reduce(
                        out=dst[:, c : c + 1],
                        in_=part[:, c * 4 : (c + 1) * 4].rearrange(
                            "p (a q) -> p a q", a=1
                        ),
                        axis=mybir.AxisListType.X,
                        op=ALU.add,
                    )
            se_q = se_fl.rearrange("p (c q) -> p c q", q=4)
            nc.vector.tensor_reduce(
                out=se_s, in_=se_q, axis=mybir.AxisListType.X, op=ALU.add
            )
            nc.scalar.activation(out=lse, in_=se_s, func=AF.Ln)
            nc.vector.reciprocal(out=rs[:], in_=s_col[:])
            nc.vector.tensor_tensor(out=dots, in0=dotraw, in1=rs, op=ALU.mult)
            nc.vector.tensor_tensor(out=losses, in0=lse, in1=dots, op=ALU.subtract)
            nc.vector.tensor_reduce(
                out=lcol, in_=losses, axis=mybir.AxisListType.X, op=ALU.add
            )
            lp_ps = psmm.tile([1, NSLICE], F32, tag="mm")
            nc.tensor.matmul(
                lp_ps[:1, :1], ones_f[:, :1], lcol[:, :1], start=True, stop=True
            )
            nc.scalar.activation(
                out=loss_sb[:1, 0:1], in_=lp_ps[:1, :1], func=AF.Copy,
                scale=loss_scale,
            )
            nc.sync.dma_start(out=out_ext[:], in_=loss_sb[:1, 0:1])

    nc.compile()
    return nc


LAST_RESULT = None


def kernel(features, prototypes, logits):
    from concourse.bass_utils import run_bass_kernel_spmd

    global LAST_RESULT
    n_cores = 8
    B, D = features.shape
    K = prototypes.shape[0]
    B_loc = B // n_cores

    nc = build_nc(B_loc=B_loc, K=K, D=D, n_cores=n_cores)

    features = np.ascontiguousarray(features, dtype=np.float32)
    prototypes = np.ascontiguousarray(prototypes, dtype=np.float32)
    logits = np.ascontiguousarray(logits, dtype=np.float32)

    in_maps = [
        {
            "features": features[i * B_loc : (i + 1) * B_loc],
            "prototypes": prototypes,
            "logits": logits[i * B_loc : (i + 1) * B_loc],
        }
        for i in range(n_cores)
    ]
    res = run_bass_kernel_spmd(
        nc,
        in_maps,
        list(range(n_cores)),
        trace=bool(os.environ.get("CLIP_OT_TRACE")),
    )
    LAST_RESULT = res
    total = 0.0
    for i in range(n_cores):
        total += float(np.asarray(res.results[i]["out"], dtype=np.float64)[0])
    return np.float32(total)
